# revision 32
# baseline (speedup 1.0000x reference)
"""Trainium2 Bass kernel for nn_ErbDecoderStep (GRU + grouped-linear + separable-conv decoder).

Strategy:
  - Pure data parallel: batch B=16384 sharded as 2048 rows per NeuronCore (8 cores).
  - All device compute is feature-major: activations live in SBUF as (features, batch)
    tiles so every matmul contracts over the partition axis. The host pre-transposes
    (and casts to bf16) the inputs and transposes the two outputs back.
  - Depthwise+pointwise+folded-BN of every separable conv block is combined on the host
    into one small banded matrix per block (transpose convs become banded upsampling
    matrices); residual adds are folded into the following matmul as an extra
    contraction block reusing the same weight tiles, or done on DVE.
  - bf16 matmuls with fp32 PSUM accumulation; bf16 DRAM outputs upcast on host.
  - Hand-pipelined schedule: per-chunk GRU groups (PSUM bank released per gate via
    [rk,rk,k,k]+sigmoid), gl_out skewed two chunks later so the gate chain drains off
    the PE critical path, then stage-major conv rounds (t3/t2/t1/fin across chunks).
  - DMA: weights/bias on the ACT HWDGE ring, emb/state chunk-sliced on the SP ring,
    e3..e0 via SWDGE; stores ride SP after all loads (HWDGE rings are FIFO - a store
    that waits on compute must never sit ahead of a load).

Self-contained: hardcodes shapes from the problem spec (B=16384, H=256, ch=16, G=8).
"""
import contextlib
import os
import sys

import numpy as np

for _p in ('/opt/trn_rl_repo', '/root/.axon_site/_ro/trn_rl_repo'):
    if _p not in sys.path and os.path.isdir(_p):
        sys.path.insert(0, _p)

import ml_dtypes  # noqa: E402

import concourse.bacc as bacc  # noqa: E402
import concourse.bass as bass  # noqa: E402
import concourse.mybir as mybir  # noqa: E402
import concourse.tile as tile  # noqa: E402
from concourse.bass_utils import run_bass_kernel_spmd  # noqa: E402

BF16 = mybir.dt.bfloat16
F32 = mybir.dt.float32
AF = mybir.ActivationFunctionType
ALU = mybir.AluOpType

B = 16384
NCORES = 8
BC = B // NCORES          # 2048 batch rows per core
NCHUNK = 4
NB = BC // NCHUNK         # 512 batch columns per chunk (one PSUM bank per f32 tile)
CH = 16

LAST_RESULTS = None       # test.py reads profiling info from here

# ---------------------------------------------------------------------------
# Host-side weight preprocessing
# ---------------------------------------------------------------------------


def _sep_mats(dw, pw, g, b, F_in, mode):
    """Combined depthwise+pointwise+BN-gain matrix (F_in*ch, F_out*ch) + bias vec."""
    dw = np.asarray(dw, np.float32)
    pw = np.asarray(pw, np.float32)
    g = np.asarray(g, np.float32)
    b = np.asarray(b, np.float32)
    kf = dw.shape[1]
    ch = dw.shape[3]
    dwv = dw[0, :, 0, :]
    pwm = pw[0, 0]
    if mode == 'same':
        F_out = F_in
        M = np.zeros((F_in * ch, F_out * ch), np.float32)
        off = 1 if kf == 3 else 0
        for fo in range(F_out):
            for t in range(kf):
                fi = fo + t - off
                if 0 <= fi < F_in:
                    M[fi * ch:(fi + 1) * ch, fo * ch:(fo + 1) * ch] += dwv[t][:, None] * pwm
    else:  # 'tr': lhs_dilation 2, pad (1,2), kf=3
        F_out = 2 * F_in
        M = np.zeros((F_in * ch, F_out * ch), np.float32)
        for fo in range(F_out):
            for t in range(3):
                j = fo + t - 1
                if j >= 0 and j % 2 == 0 and j // 2 < F_in:
                    fi = j // 2
                    M[fi * ch:(fi + 1) * ch, fo * ch:(fo + 1) * ch] += dwv[t][:, None] * pwm
    M = M * np.tile(g, F_out)[None, :]
    return M, np.tile(b, F_out)


def _final_mat(out_w, out_g, out_b, F=32):
    out_w = np.asarray(out_w, np.float32)
    ch = out_w.shape[2]
    M = np.zeros((F * ch, F), np.float32)
    for fo in range(F):
        for t in range(3):
            fi = fo + t - 1
            if 0 <= fi < F:
                M[fi * ch:(fi + 1) * ch, fo] += out_w[0, t, :, 0]
    M = M * float(np.asarray(out_g).reshape(-1)[0])
    return M, np.full((F,), float(np.asarray(out_b).reshape(-1)[0]), np.float32)


def _grouped_bd(w):
    w = np.asarray(w, np.float32)
    gn, ig, og = w.shape
    M = np.zeros((gn * ig, gn * og), np.float32)
    for gi in range(gn):
        M[gi * ig:(gi + 1) * ig, gi * og:(gi + 1) * og] = w[gi]
    return M


def _tiles_of(M):
    """Split (K, Mo) into 128x<=128 lhsT tiles, dropping all-zero tiles."""
    K, Mo = M.shape
    out = []
    for kc in range(0, K, 128):
        for mc in range(0, Mo, 128):
            t = M[kc:kc + 128, mc:mc + 128]
            if not np.any(t):
                continue
            out.append((kc // 128, mc // 128, np.ascontiguousarray(t)))
    return out


class _WeightPack:
    def __init__(self):
        self.wcols = []
        self.woff = 0
        self.wmap = {}     # name -> list of (kc, mc, col_offset, width)
        self.bcols = []
        self.bmap = {}     # name -> list of column indices (one per 128-row tile)

    def add_w(self, name, tiles):
        lst = []
        for kc, mc, t in tiles:
            k, m = t.shape
            buf = np.zeros((128, m), np.float32)
            buf[:k] = t
            lst.append((kc, mc, self.woff, m))
            self.wcols.append(buf)
            self.woff += m
        self.wmap[name] = lst

    def add_b(self, name, vec):
        vec = np.asarray(vec, np.float32).reshape(-1)
        if not np.any(vec):
            self.bmap[name] = [None] * ((len(vec) + 127) // 128)
            return
        cols = []
        for mc in range(0, len(vec), 128):
            seg = vec[mc:mc + 128]
            buf = np.zeros((128,), np.float32)
            buf[:len(seg)] = seg
            cols.append(len(self.bcols))
            self.bcols.append(buf)
        self.bmap[name] = cols

    def finalize(self):
        wts = np.concatenate(self.wcols, axis=1).astype(ml_dtypes.bfloat16)
        if self.bcols:
            bias = np.stack(self.bcols, axis=1).astype(np.float32)
        else:
            bias = np.zeros((128, 1), np.float32)
        return wts, bias


def _prepare(params):
    p = {k: np.asarray(v, np.float32) for k, v in params.items()}
    pk = _WeightPack()

    pk.add_w('gl_in', _tiles_of(_grouped_bd(p['w_in'])))
    pk.add_w('k', _tiles_of(p['gru_k']))
    pk.add_w('rk', _tiles_of(p['gru_rk']))
    pk.add_w('gl_out', _tiles_of(_grouped_bd(p['w_out'])))

    b0, b1 = p['gru_b'][0], p['gru_b'][1]
    pk.add_b('bzr', (b0 + b1)[0:512])
    pk.add_b('b0n', b0[512:768])
    pk.add_b('b1n', b1[512:768])

    for name, fin, mode in [('c3p', 8, 'same'), ('t3', 8, 'same'),
                            ('c2p', 8, 'same'), ('t2', 8, 'tr'),
                            ('c1p', 16, 'same'), ('t1', 16, 'tr'),
                            ('c0p', 32, 'same')]:
        M, bv = _sep_mats(p[name + '_dw'], p[name + '_pw'], p[name + '_g'],
                          p[name + '_b'], fin, mode)
        pk.add_w(name, _tiles_of(M))
        pk.add_b(name, bv)

    Mf, bf = _final_mat(p['out_w'], p['out_g'], p['out_b'], 32)
    pk.add_w('fin', _tiles_of(Mf))
    pk.add_b('fin', bf)
    pk.glin_wcols = max(off + m for (_, _, off, m) in pk.wmap['gl_in'])
    pk.gru_wcols = max(off + m for nm in ('gl_in', 'k', 'rk', 'gl_out')
                       for (_, _, off, m) in pk.wmap[nm])
    return pk


# ---------------------------------------------------------------------------
# Device kernel
# ---------------------------------------------------------------------------

def _build_and_emit(pk):
    nc = bacc.Bacc('TRN2', target_bir_lowering=False, debug=False,
                   num_devices=NCORES)

    dr = {}
    for name, shape, dt, kind in [
            ('embT', [256, BC], BF16, 'ExternalInput'),
            ('stateT', [256, BC], BF16, 'ExternalInput'),
            ('e3T', [128, BC], BF16, 'ExternalInput'),
            ('e2T', [128, BC], BF16, 'ExternalInput'),
            ('e1T', [256, BC], BF16, 'ExternalInput'),
            ('e0T', [512, BC], BF16, 'ExternalInput'),
            ('wts', [128, pk.woff], BF16, 'ExternalInput'),
            ('bias', [128, max(len(pk.bcols), 1)], F32, 'ExternalInput'),
            ('mT', [32, BC], BF16, 'ExternalOutput'),
            ('newstT', [256, BC], BF16, 'ExternalOutput')]:
        dr[name] = nc.dram_tensor(name, shape, dt, kind=kind).ap()

    wmap, bmap = pk.wmap, pk.bmap

    with contextlib.ExitStack() as ctx:
        tc = ctx.enter_context(tile.TileContext(nc))
        consts = ctx.enter_context(tc.tile_pool(name='consts', bufs=1))
        inp = ctx.enter_context(tc.tile_pool(name='inp', bufs=1))
        work = ctx.enter_context(tc.tile_pool(name='work', bufs=2))
        pers = ctx.enter_context(tc.tile_pool(name='pers', bufs=NCHUNK))
        outp = ctx.enter_context(tc.tile_pool(name='outp', bufs=1))
        psum = ctx.enter_context(tc.tile_pool(name='psum', bufs=4, space='PSUM'))
        cpsum = ctx.enter_context(tc.tile_pool(name='cpsum', bufs=4, space='PSUM'))

        # ---- constants ----
        nbias = max(len(pk.bcols), 1)
        wts = consts.tile([128, pk.woff], BF16, tag='wts', name='wts_sb')
        nc.scalar.dma_start(wts[:, 0:pk.glin_wcols], dr['wts'][:, 0:pk.glin_wcols])
        nc.scalar.dma_start(wts[:, pk.glin_wcols:pk.gru_wcols],
                            dr['wts'][:, pk.glin_wcols:pk.gru_wcols])
        nc.scalar.dma_start(wts[:, pk.gru_wcols:], dr['wts'][:, pk.gru_wcols:])
        bias = consts.tile([128, nbias], F32, tag='bias', name='bias_sb')
        if pk.bcols:
            nc.scalar.dma_start(bias[:], dr['bias'][:])

        def Wap(name, idx):
            kc, mc, off, m = wmap[name][idx]
            return wts[:, off:off + m]

        def Bv(name, i):
            j = bmap[name][i]
            if j is None:
                return None
            return bias[:, j:j + 1]

        def act(out, in_, func, bname=None, bi=0, psl=None):
            bv = Bv(bname, bi) if bname is not None else None
            if bv is None:
                nc.scalar.activation(out, in_, func)
            else:
                nc.scalar.activation(out, in_, func,
                                     bias=bv if psl is None else bv[psl])

        # ---- load inputs (feature-major bf16) ----
        ins = {}
        for name, nrow in [('embT', 256), ('stateT', 256), ('e3T', 128),
                           ('e2T', 128), ('e1T', 256), ('e0T', 512)]:
            ins[name] = [inp.tile([128, BC], BF16, tag=f'{name}{t}', name=f'{name}{t}')
                         for t in range(nrow // 128)]
        for c in range(NCHUNK):
            S = slice(c * NB, (c + 1) * NB)
            for name in ('embT', 'stateT'):
                for t, tl in enumerate(ins[name]):
                    nc.sync.dma_start(tl[:, S], dr[name][t * 128:(t + 1) * 128, S])
        for name in ('e3T', 'e2T', 'e1T', 'e0T'):
            for t, tl in enumerate(ins[name]):
                nc.gpsimd.dma_start(tl[:], dr[name][t * 128:(t + 1) * 128, :])

        # ---- PE warmup: dummy matmuls during the DMA head keep the HAM
        # clock-gate at 8/8 so the real matmul stream starts warm ----
        wz = consts.tile([128, NB], BF16, tag='wz', name='wz')
        nc.vector.memset(wz[:], 0.0)
        ps_w = cpsum.tile([128, NB], F32, tag='cps', name='cps')
        for i in range(20):
            nc.tensor.matmul(ps_w[:, 0:128], wz[:, 0:128], wz[:, 0:128],
                             start=i == 0, stop=i == 19)

        # ---- output staging ----
        st_out = [outp.tile([128, BC], BF16, tag=f'st{t}', name=f'st{t}')
                  for t in range(2)]
        m_out = outp.tile([32, BC], BF16, tag='m', name='m_out')

        def conv_mm(name, rhs_tiles, extra_rhs=None):
            tiles = wmap[name]
            by_m = {}
            for (kc, mc, off, m) in tiles:
                by_m.setdefault(mc, []).append((kc, off, m))
            pst = {}
            for mc, lst in sorted(by_m.items()):
                mwidth = lst[0][2]
                ps = cpsum.tile([mwidth, NB], F32, tag='cps', name='cps')
                seq = []
                for kc, off, m in lst:
                    seq.append((wts[:, off:off + m], rhs_tiles[kc]))
                    if extra_rhs is not None:
                        seq.append((wts[:, off:off + m], extra_rhs[kc]))
                for i, (w, rhs) in enumerate(seq):
                    nc.tensor.matmul(ps[:], w, rhs,
                                     start=i == 0, stop=i == len(seq) - 1)
                pst[mc] = ps
            return [pst[mc] for mc in sorted(pst)]

        def relu_b(name, pst, eng, pool, tagp=None):
            outs = []
            tagp = tagp or name
            for mi, ps in enumerate(pst):
                o = pool.tile([ps.shape[0], NB], BF16, tag=f'{tagp}r{mi}',
                              name=f'{tagp}r{mi}')
                bv = Bv(name, mi)
                if eng == 'act':
                    if bv is None:
                        nc.scalar.activation(o[:], ps[:], AF.Relu)
                    else:
                        nc.scalar.activation(o[:], ps[:], AF.Relu, bias=bv)
                else:
                    if bv is None:
                        nc.vector.tensor_scalar_max(o[:], ps[:], 0.0)
                    else:
                        nc.vector.tensor_scalar(o[:], ps[:], bv, 0.0,
                                                op0=ALU.add, op1=ALU.max)
                outs.append(o)
            return outs

        P = {}   # per-chunk persistent tiles

        def phase_gru(c):
            """GRU matmuls + gate chain + the four input-side conv blocks."""
            S = slice(c * NB, (c + 1) * NB)
            hT = [ins['stateT'][t][:, S] for t in range(2)]

            ps_gl = []
            for t in range(2):
                ps = psum.tile([128, NB], F32, tag='ps', name='ps')
                nc.tensor.matmul(ps[:], Wap('gl_in', t), ins['embT'][t][:, S],
                                 start=True, stop=True)
                ps_gl.append(ps)
            xin = []
            for t in range(2):
                xt = pers.tile([128, NB], BF16, tag=f'xin{t}', name=f'xin{t}', bufs=3)
                if c < 2:
                    nc.vector.tensor_scalar_max(xt[:], ps_gl[t][:], 0.0)
                else:
                    nc.scalar.activation(xt[:], ps_gl[t][:], AF.Relu)
                xin.append(xt)

            zr = []
            for mi in range(4):
                ps = psum.tile([128, NB], F32, tag='ps', name='ps')
                for kc in range(2):
                    nc.tensor.matmul(ps[:], Wap('rk', kc * 6 + mi), hT[kc],
                                     start=kc == 0, stop=False)
                for kc in range(2):
                    nc.tensor.matmul(ps[:], Wap('k', kc * 6 + mi), xin[kc][:],
                                     start=False, stop=kc == 1)
                g = work.tile([128, NB], BF16, tag=f'zr{mi}', name=f'zr{mi}')
                act(g[:], ps[:], AF.Sigmoid, 'bzr', mi)
                zr.append(g)
            z, r = zr[0:2], zr[2:4]

            zh = []
            for t in range(2):
                zht = work.tile([128, NB], BF16, tag=f'zh{t}', name=f'zh{t}')
                nc.vector.tensor_tensor(zht[:], z[t][:], hT[t], op=ALU.mult)
                zh.append(zht)

            for t in range(2):
                mi = 4 + t
                ps_hn = psum.tile([128, NB], F32, tag='ps', name='ps')
                for kc in range(2):
                    nc.tensor.matmul(ps_hn[:], Wap('rk', kc * 6 + mi), hT[kc],
                                     start=kc == 0, stop=kc == 1)
                ps_xn = psum.tile([128, NB], F32, tag='ps', name='ps')
                for kc in range(2):
                    nc.tensor.matmul(ps_xn[:], Wap('k', kc * 6 + mi), xin[kc][:],
                                     start=kc == 0, stop=kc == 1)
                hnr = work.tile([128, NB], BF16, tag=f'hnr{t}', name=f'hnr{t}')
                b1nv = Bv('b1n', t)
                if b1nv is None:
                    nc.vector.tensor_tensor(hnr[:], ps_hn[:], r[t][:], op=ALU.mult)
                else:
                    nc.vector.scalar_tensor_tensor(hnr[:], ps_hn[:], b1nv,
                                                   r[t][:], op0=ALU.add,
                                                   op1=ALU.mult)
                nin = work.tile([128, NB], BF16, tag=f'nin{t}', name=f'nin{t}')
                nc.vector.tensor_tensor(nin[:], ps_xn[:], hnr[:], op=ALU.add)
                nt = work.tile([128, NB], BF16, tag=f'n{t}', name=f'n{t}')
                act(nt[:], nin[:], AF.Tanh, 'b0n', t)
                zn = work.tile([128, NB], BF16, tag=f'zn{t}', name=f'zn{t}')
                nc.vector.tensor_tensor(zn[:], z[t][:], nt[:], op=ALU.mult)
                nmz = work.tile([128, NB], BF16, tag=f'nmz{t}', name=f'nmz{t}')
                nc.vector.tensor_tensor(nmz[:], nt[:], zn[:], op=ALU.subtract)
                hpt = st_out[t][:, S]
                nc.vector.tensor_tensor(hpt, zh[t][:], nmz[:], op=ALU.add)

            for t in range(2):
                nc.sync.dma_start(dr['newstT'][t * 128:(t + 1) * 128, S],
                                  st_out[t][:, S])

            # input-side conv blocks (only need e*T): PE filler for gate drain
            rc3 = relu_b('c3p', conv_mm('c3p', [ins['e3T'][0][:, S]]), 'vec', pers)
            rc2 = relu_b('c2p', conv_mm('c2p', [ins['e2T'][0][:, S]]), 'vec', pers)
            rc1 = relu_b('c1p', conv_mm('c1p', [t[:, S] for t in ins['e1T']]),
                         'vec', pers)
            rc0 = relu_b('c0p', conv_mm('c0p', [t[:, S] for t in ins['e0T']]),
                         'act', pers)
            P[c] = dict(xin=xin, rc3=rc3, rc2=rc2, rc1=rc1, rc0=rc0)

        def phase_glout(c):
            """gl_out (gout = hp + xin folded as extra contraction) -> emb4."""
            S = slice(c * NB, (c + 1) * NB)
            ps = cpsum.tile([128, NB], F32, tag='cps', name='cps')
            seqg = []
            for i in range(2):
                kc = wmap['gl_out'][i][0]
                seqg.append((Wap('gl_out', i), st_out[kc][:, S]))
            for i in range(2):
                kc = wmap['gl_out'][i][0]
                seqg.append((Wap('gl_out', i), P[c]['xin'][kc][:]))
            for i, (w, rhs) in enumerate(seqg):
                nc.tensor.matmul(ps[:], w, rhs, start=i == 0,
                                 stop=i == len(seqg) - 1)
            emb4 = pers.tile([128, NB], BF16, tag='emb4', name='emb4', bufs=3)
            nc.scalar.activation(emb4[:], ps[:], AF.Relu)
            P[c]['emb4'] = emb4

        def stage_t3(c):
            P[c]['x3'] = relu_b('t3', conv_mm('t3', [P[c]['rc3'][0][:]],
                                              extra_rhs=[P[c]['emb4'][:]]),
                                'act', pers)

        def stage_t2(c):
            P[c]['x2'] = relu_b('t2', conv_mm('t2', [P[c]['rc2'][0][:]],
                                              extra_rhs=[P[c]['x3'][0][:]]),
                                'act', pers)

        def stage_t1(c):
            x1 = []
            for t in range(2):
                xt = work.tile([128, NB], BF16, tag=f'x1_{t}', name=f'x1_{t}')
                nc.vector.tensor_tensor(xt[:], P[c]['rc1'][t][:], P[c]['x2'][t][:],
                                        op=ALU.add)
                x1.append(xt)
            P[c]['x1t'] = relu_b('t1', conv_mm('t1', [x[:] for x in x1]),
                                 'vec', pers)

        def stage_fin(c):
            S = slice(c * NB, (c + 1) * NB)
            psf = cpsum.tile([32, NB], F32, tag='cps', name='cps')
            seqf = []
            for (kc, mc, off, m) in wmap['fin']:
                seqf.append((wts[:, off:off + m], P[c]['rc0'][kc][:]))
                seqf.append((wts[:, off:off + m], P[c]['x1t'][kc][:]))
            for i, (w, rhs) in enumerate(seqf):
                nc.tensor.matmul(psf[:], w, rhs, start=i == 0,
                                 stop=i == len(seqf) - 1)
            act(m_out[:, S], psf[:], AF.Sigmoid, 'fin', 0, psl=slice(0, 32))
            nc.sync.dma_start(dr['mT'][:, S], m_out[:, S])

        # Hand-pipelined schedule: glout(c) is skewed two GRU chunks later so
        # the gate chain of chunk c drains while PE runs chunks c+1, c+2.
        phase_gru(0)
        phase_gru(1)
        phase_glout(0)
        phase_gru(2)
        phase_glout(1)
        phase_gru(3)
        phase_glout(2)
        stage_t3(0)
        phase_glout(3)
        stage_t3(1)
        stage_t2(0)
        stage_t3(2)
        stage_t2(1)
        stage_t3(3)
        stage_t2(2)
        stage_t1(0)
        stage_t2(3)
        stage_t1(1)
        stage_fin(0)
        stage_t1(2)
        stage_fin(1)
        stage_t1(3)
        stage_fin(2)
        stage_fin(3)

    nc.compile()
    return nc


# ---------------------------------------------------------------------------
# Public entry point
# ---------------------------------------------------------------------------

def make_in_maps(emb, e3, e2, e1, e0, erb_dec_state, wts, bias):
    bf = ml_dtypes.bfloat16
    embT = np.ascontiguousarray(np.asarray(emb, np.float32).T).astype(bf)
    stT = np.ascontiguousarray(np.asarray(erb_dec_state, np.float32).T).astype(bf)
    e3T = np.ascontiguousarray(np.asarray(e3, np.float32).reshape(B, -1).T).astype(bf)
    e2T = np.ascontiguousarray(np.asarray(e2, np.float32).reshape(B, -1).T).astype(bf)
    e1T = np.ascontiguousarray(np.asarray(e1, np.float32).reshape(B, -1).T).astype(bf)
    e0T = np.ascontiguousarray(np.asarray(e0, np.float32).reshape(B, -1).T).astype(bf)
    in_maps = []
    for i in range(NCORES):
        S = slice(i * BC, (i + 1) * BC)
        in_maps.append({
            'embT': np.ascontiguousarray(embT[:, S]),
            'stateT': np.ascontiguousarray(stT[:, S]),
            'e3T': np.ascontiguousarray(e3T[:, S]),
            'e2T': np.ascontiguousarray(e2T[:, S]),
            'e1T': np.ascontiguousarray(e1T[:, S]),
            'e0T': np.ascontiguousarray(e0T[:, S]),
            'wts': wts,
            'bias': bias,
        })
    return in_maps


def _ensure_ntff_hook():
    """Register the axon NTFF profiling hook if the image's antenv lacks it."""
    try:
        from antenv.axon_hooks import get_axon_ntff_profile_hook  # noqa: F401
        return
    except ImportError:
        pass
    import types

    import antenv
    mod = types.ModuleType('antenv.axon_hooks')
    _h = [None]
    mod.get_axon_ntff_profile_hook = lambda: _h[0]
    mod.set_axon_ntff_profile_hook = lambda hook: _h.__setitem__(0, hook)
    sys.modules['antenv.axon_hooks'] = mod
    antenv.axon_hooks = mod
    try:
        from trn_agent_boot.trn_boot import _ntff_profile_via_ctypes
        mod.set_axon_ntff_profile_hook(
            _ntff_profile_via_ctypes('/opt/axon/libaxon_pjrt.so'))
    except Exception:
        pass


def kernel(emb, e3, e2, e1, e0, erb_dec_state, params):
    global LAST_RESULTS
    if os.environ.get('BASS_TRACE'):
        _ensure_ntff_hook()
    pk = _prepare(params)
    wts, bias = pk.finalize()
    nc = _build_and_emit(pk)
    in_maps = make_in_maps(emb, e3, e2, e1, e0, erb_dec_state, wts, bias)

    res = run_bass_kernel_spmd(nc, in_maps, list(range(NCORES)),
                               trace=bool(os.environ.get('BASS_TRACE')))
    LAST_RESULTS = res

    m = np.empty((B, 32), np.float32)
    new_state = np.empty((B, 256), np.float32)
    for i in range(NCORES):
        S = slice(i * BC, (i + 1) * BC)
        m[S] = np.asarray(res.results[i]['mT']).astype(np.float32).T
        new_state[S] = np.asarray(res.results[i]['newstT']).astype(np.float32).T
    return m.reshape(B, 1, 1, 32), new_state


# revision 33
# speedup vs baseline: 1.0031x; 1.0031x over previous
"""Trainium2 Bass kernel for nn_ErbDecoderStep (GRU + grouped-linear + separable-conv decoder).

Strategy:
  - Pure data parallel: batch B=16384 sharded as 2048 rows per NeuronCore (8 cores).
  - All device compute is feature-major: activations live in SBUF as (features, batch)
    tiles so every matmul contracts over the partition axis. The host pre-transposes
    (and casts to bf16) the inputs and transposes the two outputs back.
  - Depthwise+pointwise+folded-BN of every separable conv block is combined on the host
    into one small banded matrix per block (transpose convs become banded upsampling
    matrices); residual adds are folded into the following matmul as an extra
    contraction block reusing the same weight tiles, or done on DVE.
  - bf16 matmuls with fp32 PSUM accumulation; bf16 DRAM outputs upcast on host.
  - Hand-pipelined schedule: per-chunk GRU groups (PSUM bank released per gate via
    [rk,rk,k,k]+sigmoid), gl_out skewed two chunks later so the gate chain drains off
    the PE critical path, then stage-major conv rounds (t3/t2/t1/fin across chunks).
  - DMA: weights/bias on the ACT HWDGE ring, emb/state chunk-sliced on the SP ring,
    e3..e0 via SWDGE; stores ride SP after all loads (HWDGE rings are FIFO - a store
    that waits on compute must never sit ahead of a load).

Self-contained: hardcodes shapes from the problem spec (B=16384, H=256, ch=16, G=8).
"""
import contextlib
import os
import sys

import numpy as np

for _p in ('/opt/trn_rl_repo', '/root/.axon_site/_ro/trn_rl_repo'):
    if _p not in sys.path and os.path.isdir(_p):
        sys.path.insert(0, _p)

import ml_dtypes  # noqa: E402

import concourse.bacc as bacc  # noqa: E402
import concourse.bass as bass  # noqa: E402
import concourse.mybir as mybir  # noqa: E402
import concourse.tile as tile  # noqa: E402
from concourse.bass_utils import run_bass_kernel_spmd  # noqa: E402

BF16 = mybir.dt.bfloat16
F32 = mybir.dt.float32
AF = mybir.ActivationFunctionType
ALU = mybir.AluOpType

B = 16384
NCORES = 8
BC = B // NCORES          # 2048 batch rows per core
NCHUNK = 4
NB = BC // NCHUNK         # 512 batch columns per chunk (one PSUM bank per f32 tile)
CH = 16

LAST_RESULTS = None       # test.py reads profiling info from here

# ---------------------------------------------------------------------------
# Host-side weight preprocessing
# ---------------------------------------------------------------------------


def _sep_mats(dw, pw, g, b, F_in, mode):
    """Combined depthwise+pointwise+BN-gain matrix (F_in*ch, F_out*ch) + bias vec."""
    dw = np.asarray(dw, np.float32)
    pw = np.asarray(pw, np.float32)
    g = np.asarray(g, np.float32)
    b = np.asarray(b, np.float32)
    kf = dw.shape[1]
    ch = dw.shape[3]
    dwv = dw[0, :, 0, :]
    pwm = pw[0, 0]
    if mode == 'same':
        F_out = F_in
        M = np.zeros((F_in * ch, F_out * ch), np.float32)
        off = 1 if kf == 3 else 0
        for fo in range(F_out):
            for t in range(kf):
                fi = fo + t - off
                if 0 <= fi < F_in:
                    M[fi * ch:(fi + 1) * ch, fo * ch:(fo + 1) * ch] += dwv[t][:, None] * pwm
    else:  # 'tr': lhs_dilation 2, pad (1,2), kf=3
        F_out = 2 * F_in
        M = np.zeros((F_in * ch, F_out * ch), np.float32)
        for fo in range(F_out):
            for t in range(3):
                j = fo + t - 1
                if j >= 0 and j % 2 == 0 and j // 2 < F_in:
                    fi = j // 2
                    M[fi * ch:(fi + 1) * ch, fo * ch:(fo + 1) * ch] += dwv[t][:, None] * pwm
    M = M * np.tile(g, F_out)[None, :]
    return M, np.tile(b, F_out)


def _final_mat(out_w, out_g, out_b, F=32):
    out_w = np.asarray(out_w, np.float32)
    ch = out_w.shape[2]
    M = np.zeros((F * ch, F), np.float32)
    for fo in range(F):
        for t in range(3):
            fi = fo + t - 1
            if 0 <= fi < F:
                M[fi * ch:(fi + 1) * ch, fo] += out_w[0, t, :, 0]
    M = M * float(np.asarray(out_g).reshape(-1)[0])
    return M, np.full((F,), float(np.asarray(out_b).reshape(-1)[0]), np.float32)


def _grouped_bd(w):
    w = np.asarray(w, np.float32)
    gn, ig, og = w.shape
    M = np.zeros((gn * ig, gn * og), np.float32)
    for gi in range(gn):
        M[gi * ig:(gi + 1) * ig, gi * og:(gi + 1) * og] = w[gi]
    return M


def _tiles_of(M):
    """Split (K, Mo) into 128x<=128 lhsT tiles, dropping all-zero tiles."""
    K, Mo = M.shape
    out = []
    for kc in range(0, K, 128):
        for mc in range(0, Mo, 128):
            t = M[kc:kc + 128, mc:mc + 128]
            if not np.any(t):
                continue
            out.append((kc // 128, mc // 128, np.ascontiguousarray(t)))
    return out


class _WeightPack:
    def __init__(self):
        self.wcols = []
        self.woff = 0
        self.wmap = {}     # name -> list of (kc, mc, col_offset, width)
        self.bcols = []
        self.bmap = {}     # name -> list of column indices (one per 128-row tile)

    def add_w(self, name, tiles):
        lst = []
        for kc, mc, t in tiles:
            k, m = t.shape
            buf = np.zeros((128, m), np.float32)
            buf[:k] = t
            lst.append((kc, mc, self.woff, m))
            self.wcols.append(buf)
            self.woff += m
        self.wmap[name] = lst

    def add_b(self, name, vec):
        vec = np.asarray(vec, np.float32).reshape(-1)
        if not np.any(vec):
            self.bmap[name] = [None] * ((len(vec) + 127) // 128)
            return
        cols = []
        for mc in range(0, len(vec), 128):
            seg = vec[mc:mc + 128]
            buf = np.zeros((128,), np.float32)
            buf[:len(seg)] = seg
            cols.append(len(self.bcols))
            self.bcols.append(buf)
        self.bmap[name] = cols

    def finalize(self):
        wts = np.concatenate(self.wcols, axis=1).astype(ml_dtypes.bfloat16)
        if self.bcols:
            bias = np.stack(self.bcols, axis=1).astype(np.float32)
        else:
            bias = np.zeros((128, 1), np.float32)
        return wts, bias


def _prepare(params):
    p = {k: np.asarray(v, np.float32) for k, v in params.items()}
    pk = _WeightPack()

    pk.add_w('gl_in', _tiles_of(_grouped_bd(p['w_in'])))
    pk.add_w('k', _tiles_of(p['gru_k']))
    pk.add_w('rk', _tiles_of(p['gru_rk']))
    pk.add_w('gl_out', _tiles_of(_grouped_bd(p['w_out'])))

    b0, b1 = p['gru_b'][0], p['gru_b'][1]
    pk.add_b('bzr', (b0 + b1)[0:512])
    pk.add_b('b0n', b0[512:768])
    pk.add_b('b1n', b1[512:768])

    for name, fin, mode in [('c3p', 8, 'same'), ('t3', 8, 'same'),
                            ('c2p', 8, 'same'), ('t2', 8, 'tr'),
                            ('c1p', 16, 'same'), ('t1', 16, 'tr'),
                            ('c0p', 32, 'same')]:
        M, bv = _sep_mats(p[name + '_dw'], p[name + '_pw'], p[name + '_g'],
                          p[name + '_b'], fin, mode)
        pk.add_w(name, _tiles_of(M))
        pk.add_b(name, bv)

    Mf, bf = _final_mat(p['out_w'], p['out_g'], p['out_b'], 32)
    pk.add_w('fin', _tiles_of(Mf))
    pk.add_b('fin', bf)
    pk.glin_wcols = max(off + m for (_, _, off, m) in pk.wmap['gl_in'])
    pk.gru_wcols = max(off + m for nm in ('gl_in', 'k', 'rk', 'gl_out')
                       for (_, _, off, m) in pk.wmap[nm])
    return pk


# ---------------------------------------------------------------------------
# Device kernel
# ---------------------------------------------------------------------------

def _build_and_emit(pk):
    nc = bacc.Bacc('TRN2', target_bir_lowering=False, debug=False,
                   num_devices=NCORES)

    dr = {}
    for name, shape, dt, kind in [
            ('embT', [256, BC], BF16, 'ExternalInput'),
            ('stateT', [256, BC], BF16, 'ExternalInput'),
            ('e3T', [128, BC], BF16, 'ExternalInput'),
            ('e2T', [128, BC], BF16, 'ExternalInput'),
            ('e1T', [256, BC], BF16, 'ExternalInput'),
            ('e0T', [512, BC], BF16, 'ExternalInput'),
            ('wts', [128, pk.woff], BF16, 'ExternalInput'),
            ('bias', [128, max(len(pk.bcols), 1)], F32, 'ExternalInput'),
            ('mT', [32, BC], BF16, 'ExternalOutput'),
            ('newstT', [256, BC], BF16, 'ExternalOutput')]:
        dr[name] = nc.dram_tensor(name, shape, dt, kind=kind).ap()

    wmap, bmap = pk.wmap, pk.bmap

    with contextlib.ExitStack() as ctx:
        tc = ctx.enter_context(tile.TileContext(nc))
        consts = ctx.enter_context(tc.tile_pool(name='consts', bufs=1))
        inp = ctx.enter_context(tc.tile_pool(name='inp', bufs=1))
        work = ctx.enter_context(tc.tile_pool(name='work', bufs=2))
        pers = ctx.enter_context(tc.tile_pool(name='pers', bufs=NCHUNK))
        outp = ctx.enter_context(tc.tile_pool(name='outp', bufs=1))
        psum = ctx.enter_context(tc.tile_pool(name='psum', bufs=4, space='PSUM'))
        cpsum = ctx.enter_context(tc.tile_pool(name='cpsum', bufs=4, space='PSUM'))

        # ---- constants ----
        nbias = max(len(pk.bcols), 1)
        wts = consts.tile([128, pk.woff], BF16, tag='wts', name='wts_sb')
        nc.scalar.dma_start(wts[:, 0:pk.glin_wcols], dr['wts'][:, 0:pk.glin_wcols])
        nc.scalar.dma_start(wts[:, pk.glin_wcols:pk.gru_wcols],
                            dr['wts'][:, pk.glin_wcols:pk.gru_wcols])
        nc.scalar.dma_start(wts[:, pk.gru_wcols:], dr['wts'][:, pk.gru_wcols:])
        bias = consts.tile([128, nbias], F32, tag='bias', name='bias_sb')
        if pk.bcols:
            nc.scalar.dma_start(bias[:], dr['bias'][:])

        def Wap(name, idx):
            kc, mc, off, m = wmap[name][idx]
            return wts[:, off:off + m]

        def Bv(name, i):
            j = bmap[name][i]
            if j is None:
                return None
            return bias[:, j:j + 1]

        def act(out, in_, func, bname=None, bi=0, psl=None):
            bv = Bv(bname, bi) if bname is not None else None
            if bv is None:
                nc.scalar.activation(out, in_, func)
            else:
                nc.scalar.activation(out, in_, func,
                                     bias=bv if psl is None else bv[psl])

        # ---- load inputs (feature-major bf16) ----
        ins = {}
        for name, nrow in [('embT', 256), ('stateT', 256), ('e3T', 128),
                           ('e2T', 128), ('e1T', 256), ('e0T', 512)]:
            ins[name] = [inp.tile([128, BC], BF16, tag=f'{name}{t}', name=f'{name}{t}')
                         for t in range(nrow // 128)]
        for c in range(NCHUNK):
            S = slice(c * NB, (c + 1) * NB)
            for name in ('embT', 'stateT'):
                for t, tl in enumerate(ins[name]):
                    nc.sync.dma_start(tl[:, S], dr[name][t * 128:(t + 1) * 128, S])
        for name in ('e3T', 'e2T', 'e1T', 'e0T'):
            for t, tl in enumerate(ins[name]):
                nc.gpsimd.dma_start(tl[:], dr[name][t * 128:(t + 1) * 128, :])

        # ---- PE warmup: dummy matmuls during the DMA head keep the HAM
        # clock-gate at 8/8 so the real matmul stream starts warm ----
        wz = consts.tile([128, NB], BF16, tag='wz', name='wz')
        nc.vector.memset(wz[:], 0.0)
        ps_w = cpsum.tile([128, NB], F32, tag='cps', name='cps')
        for i in range(24):
            nc.tensor.matmul(ps_w[:], wz[:, 0:128], wz[:], start=i == 0, stop=i == 23)

        # ---- output staging ----
        st_out = [outp.tile([128, BC], BF16, tag=f'st{t}', name=f'st{t}')
                  for t in range(2)]
        m_out = outp.tile([32, BC], BF16, tag='m', name='m_out')

        def conv_mm(name, rhs_tiles, extra_rhs=None):
            tiles = wmap[name]
            by_m = {}
            for (kc, mc, off, m) in tiles:
                by_m.setdefault(mc, []).append((kc, off, m))
            pst = {}
            for mc, lst in sorted(by_m.items()):
                mwidth = lst[0][2]
                ps = cpsum.tile([mwidth, NB], F32, tag='cps', name='cps')
                seq = []
                for kc, off, m in lst:
                    seq.append((wts[:, off:off + m], rhs_tiles[kc]))
                    if extra_rhs is not None:
                        seq.append((wts[:, off:off + m], extra_rhs[kc]))
                for i, (w, rhs) in enumerate(seq):
                    nc.tensor.matmul(ps[:], w, rhs,
                                     start=i == 0, stop=i == len(seq) - 1)
                pst[mc] = ps
            return [pst[mc] for mc in sorted(pst)]

        def relu_b(name, pst, eng, pool, tagp=None):
            outs = []
            tagp = tagp or name
            for mi, ps in enumerate(pst):
                o = pool.tile([ps.shape[0], NB], BF16, tag=f'{tagp}r{mi}',
                              name=f'{tagp}r{mi}')
                bv = Bv(name, mi)
                if eng == 'act':
                    if bv is None:
                        nc.scalar.activation(o[:], ps[:], AF.Relu)
                    else:
                        nc.scalar.activation(o[:], ps[:], AF.Relu, bias=bv)
                else:
                    if bv is None:
                        nc.vector.tensor_scalar_max(o[:], ps[:], 0.0)
                    else:
                        nc.vector.tensor_scalar(o[:], ps[:], bv, 0.0,
                                                op0=ALU.add, op1=ALU.max)
                outs.append(o)
            return outs

        P = {}   # per-chunk persistent tiles

        def phase_gru(c):
            """GRU matmuls + gate chain + the four input-side conv blocks."""
            S = slice(c * NB, (c + 1) * NB)
            hT = [ins['stateT'][t][:, S] for t in range(2)]

            ps_gl = []
            for t in range(2):
                ps = psum.tile([128, NB], F32, tag='ps', name='ps')
                nc.tensor.matmul(ps[:], Wap('gl_in', t), ins['embT'][t][:, S],
                                 start=True, stop=True)
                ps_gl.append(ps)
            xin = []
            for t in range(2):
                xt = pers.tile([128, NB], BF16, tag=f'xin{t}', name=f'xin{t}', bufs=3)
                nc.scalar.activation(xt[:], ps_gl[t][:], AF.Relu)
                xin.append(xt)

            zr = []
            for mi in range(4):
                ps = psum.tile([128, NB], F32, tag='ps', name='ps')
                for kc in range(2):
                    nc.tensor.matmul(ps[:], Wap('rk', kc * 6 + mi), hT[kc],
                                     start=kc == 0, stop=False)
                for kc in range(2):
                    nc.tensor.matmul(ps[:], Wap('k', kc * 6 + mi), xin[kc][:],
                                     start=False, stop=kc == 1)
                g = work.tile([128, NB], BF16, tag=f'zr{mi}', name=f'zr{mi}')
                act(g[:], ps[:], AF.Sigmoid, 'bzr', mi)
                zr.append(g)
            z, r = zr[0:2], zr[2:4]

            zh = []
            for t in range(2):
                zht = work.tile([128, NB], BF16, tag=f'zh{t}', name=f'zh{t}')
                nc.vector.tensor_tensor(zht[:], z[t][:], hT[t], op=ALU.mult)
                zh.append(zht)

            for t in range(2):
                mi = 4 + t
                ps_hn = psum.tile([128, NB], F32, tag='ps', name='ps')
                for kc in range(2):
                    nc.tensor.matmul(ps_hn[:], Wap('rk', kc * 6 + mi), hT[kc],
                                     start=kc == 0, stop=kc == 1)
                ps_xn = psum.tile([128, NB], F32, tag='ps', name='ps')
                for kc in range(2):
                    nc.tensor.matmul(ps_xn[:], Wap('k', kc * 6 + mi), xin[kc][:],
                                     start=kc == 0, stop=kc == 1)
                hnr = work.tile([128, NB], BF16, tag=f'hnr{t}', name=f'hnr{t}')
                b1nv = Bv('b1n', t)
                if b1nv is None:
                    nc.vector.tensor_tensor(hnr[:], ps_hn[:], r[t][:], op=ALU.mult)
                else:
                    nc.vector.scalar_tensor_tensor(hnr[:], ps_hn[:], b1nv,
                                                   r[t][:], op0=ALU.add,
                                                   op1=ALU.mult)
                nin = work.tile([128, NB], BF16, tag=f'nin{t}', name=f'nin{t}')
                nc.vector.tensor_tensor(nin[:], ps_xn[:], hnr[:], op=ALU.add)
                nt = work.tile([128, NB], BF16, tag=f'n{t}', name=f'n{t}')
                act(nt[:], nin[:], AF.Tanh, 'b0n', t)
                zn = work.tile([128, NB], BF16, tag=f'zn{t}', name=f'zn{t}')
                nc.vector.tensor_tensor(zn[:], z[t][:], nt[:], op=ALU.mult)
                nmz = work.tile([128, NB], BF16, tag=f'nmz{t}', name=f'nmz{t}')
                nc.vector.tensor_tensor(nmz[:], nt[:], zn[:], op=ALU.subtract)
                hpt = st_out[t][:, S]
                nc.vector.tensor_tensor(hpt, zh[t][:], nmz[:], op=ALU.add)

            for t in range(2):
                nc.sync.dma_start(dr['newstT'][t * 128:(t + 1) * 128, S],
                                  st_out[t][:, S])

            # input-side conv blocks (only need e*T): PE filler for gate drain
            rc3 = relu_b('c3p', conv_mm('c3p', [ins['e3T'][0][:, S]]), 'vec', pers)
            rc2 = relu_b('c2p', conv_mm('c2p', [ins['e2T'][0][:, S]]), 'vec', pers)
            rc1 = relu_b('c1p', conv_mm('c1p', [t[:, S] for t in ins['e1T']]),
                         'vec', pers)
            rc0 = relu_b('c0p', conv_mm('c0p', [t[:, S] for t in ins['e0T']]),
                         'act', pers)
            P[c] = dict(xin=xin, rc3=rc3, rc2=rc2, rc1=rc1, rc0=rc0)

        def phase_glout(c):
            """gl_out (gout = hp + xin folded as extra contraction) -> emb4."""
            S = slice(c * NB, (c + 1) * NB)
            ps = cpsum.tile([128, NB], F32, tag='cps', name='cps')
            seqg = []
            for i in range(2):
                kc = wmap['gl_out'][i][0]
                seqg.append((Wap('gl_out', i), st_out[kc][:, S]))
            for i in range(2):
                kc = wmap['gl_out'][i][0]
                seqg.append((Wap('gl_out', i), P[c]['xin'][kc][:]))
            for i, (w, rhs) in enumerate(seqg):
                nc.tensor.matmul(ps[:], w, rhs, start=i == 0,
                                 stop=i == len(seqg) - 1)
            emb4 = pers.tile([128, NB], BF16, tag='emb4', name='emb4', bufs=3)
            nc.scalar.activation(emb4[:], ps[:], AF.Relu)
            P[c]['emb4'] = emb4

        def stage_t3(c):
            P[c]['x3'] = relu_b('t3', conv_mm('t3', [P[c]['rc3'][0][:]],
                                              extra_rhs=[P[c]['emb4'][:]]),
                                'act', pers)

        def stage_t2(c):
            P[c]['x2'] = relu_b('t2', conv_mm('t2', [P[c]['rc2'][0][:]],
                                              extra_rhs=[P[c]['x3'][0][:]]),
                                'act', pers)

        def stage_t1(c):
            x1 = []
            for t in range(2):
                xt = work.tile([128, NB], BF16, tag=f'x1_{t}', name=f'x1_{t}')
                nc.vector.tensor_tensor(xt[:], P[c]['rc1'][t][:], P[c]['x2'][t][:],
                                        op=ALU.add)
                x1.append(xt)
            P[c]['x1t'] = relu_b('t1', conv_mm('t1', [x[:] for x in x1]),
                                 'vec', pers)

        def stage_fin(c):
            S = slice(c * NB, (c + 1) * NB)
            psf = cpsum.tile([32, NB], F32, tag='cps', name='cps')
            seqf = []
            for (kc, mc, off, m) in wmap['fin']:
                seqf.append((wts[:, off:off + m], P[c]['rc0'][kc][:]))
                seqf.append((wts[:, off:off + m], P[c]['x1t'][kc][:]))
            for i, (w, rhs) in enumerate(seqf):
                nc.tensor.matmul(psf[:], w, rhs, start=i == 0,
                                 stop=i == len(seqf) - 1)
            act(m_out[:, S], psf[:], AF.Sigmoid, 'fin', 0, psl=slice(0, 32))
            nc.sync.dma_start(dr['mT'][:, S], m_out[:, S])

        # Hand-pipelined schedule: glout(c) is skewed two GRU chunks later so
        # the gate chain of chunk c drains while PE runs chunks c+1, c+2.
        phase_gru(0)
        phase_gru(1)
        phase_glout(0)
        phase_gru(2)
        phase_glout(1)
        phase_gru(3)
        phase_glout(2)
        stage_t3(0)
        phase_glout(3)
        stage_t3(1)
        stage_t2(0)
        stage_t3(2)
        stage_t2(1)
        stage_t3(3)
        stage_t2(2)
        stage_t1(0)
        stage_t2(3)
        stage_t1(1)
        stage_fin(0)
        stage_t1(2)
        stage_fin(1)
        stage_t1(3)
        stage_fin(2)
        stage_fin(3)

    nc.compile()
    return nc


# ---------------------------------------------------------------------------
# Public entry point
# ---------------------------------------------------------------------------

def make_in_maps(emb, e3, e2, e1, e0, erb_dec_state, wts, bias):
    bf = ml_dtypes.bfloat16
    embT = np.ascontiguousarray(np.asarray(emb, np.float32).T).astype(bf)
    stT = np.ascontiguousarray(np.asarray(erb_dec_state, np.float32).T).astype(bf)
    e3T = np.ascontiguousarray(np.asarray(e3, np.float32).reshape(B, -1).T).astype(bf)
    e2T = np.ascontiguousarray(np.asarray(e2, np.float32).reshape(B, -1).T).astype(bf)
    e1T = np.ascontiguousarray(np.asarray(e1, np.float32).reshape(B, -1).T).astype(bf)
    e0T = np.ascontiguousarray(np.asarray(e0, np.float32).reshape(B, -1).T).astype(bf)
    in_maps = []
    for i in range(NCORES):
        S = slice(i * BC, (i + 1) * BC)
        in_maps.append({
            'embT': np.ascontiguousarray(embT[:, S]),
            'stateT': np.ascontiguousarray(stT[:, S]),
            'e3T': np.ascontiguousarray(e3T[:, S]),
            'e2T': np.ascontiguousarray(e2T[:, S]),
            'e1T': np.ascontiguousarray(e1T[:, S]),
            'e0T': np.ascontiguousarray(e0T[:, S]),
            'wts': wts,
            'bias': bias,
        })
    return in_maps


def _ensure_ntff_hook():
    """Register the axon NTFF profiling hook if the image's antenv lacks it."""
    try:
        from antenv.axon_hooks import get_axon_ntff_profile_hook  # noqa: F401
        return
    except ImportError:
        pass
    import types

    import antenv
    mod = types.ModuleType('antenv.axon_hooks')
    _h = [None]
    mod.get_axon_ntff_profile_hook = lambda: _h[0]
    mod.set_axon_ntff_profile_hook = lambda hook: _h.__setitem__(0, hook)
    sys.modules['antenv.axon_hooks'] = mod
    antenv.axon_hooks = mod
    try:
        from trn_agent_boot.trn_boot import _ntff_profile_via_ctypes
        mod.set_axon_ntff_profile_hook(
            _ntff_profile_via_ctypes('/opt/axon/libaxon_pjrt.so'))
    except Exception:
        pass


def kernel(emb, e3, e2, e1, e0, erb_dec_state, params):
    global LAST_RESULTS
    if os.environ.get('BASS_TRACE'):
        _ensure_ntff_hook()
    pk = _prepare(params)
    wts, bias = pk.finalize()
    nc = _build_and_emit(pk)
    in_maps = make_in_maps(emb, e3, e2, e1, e0, erb_dec_state, wts, bias)

    res = run_bass_kernel_spmd(nc, in_maps, list(range(NCORES)),
                               trace=bool(os.environ.get('BASS_TRACE')))
    LAST_RESULTS = res

    m = np.empty((B, 32), np.float32)
    new_state = np.empty((B, 256), np.float32)
    for i in range(NCORES):
        S = slice(i * BC, (i + 1) * BC)
        m[S] = np.asarray(res.results[i]['mT']).astype(np.float32).T
        new_state[S] = np.asarray(res.results[i]['newstT']).astype(np.float32).T
    return m.reshape(B, 1, 1, 32), new_state


# revision 34
# speedup vs baseline: 1.0146x; 1.0114x over previous
"""Trainium2 Bass kernel for nn_ErbDecoderStep (GRU + grouped-linear + separable-conv decoder).

Strategy:
  - Pure data parallel: batch B=16384 sharded as 2048 rows per NeuronCore (8 cores).
  - All device compute is feature-major: activations live in SBUF as (features, batch)
    tiles so every matmul contracts over the partition axis. The host pre-transposes
    (and casts to bf16) the inputs and transposes the two outputs back.
  - Depthwise+pointwise+folded-BN of every separable conv block is combined on the host
    into one small banded matrix per block (transpose convs become banded upsampling
    matrices); residual adds are folded into the following matmul as an extra
    contraction block reusing the same weight tiles, or done on DVE.
  - bf16 matmuls with fp32 PSUM accumulation; bf16 DRAM outputs upcast on host.
  - Hand-pipelined schedule: per-chunk GRU groups (PSUM bank released per gate via
    [rk,rk,k,k]+sigmoid), gl_out skewed two chunks later so the gate chain drains off
    the PE critical path, then stage-major conv rounds (t3/t2/t1/fin across chunks).
  - DMA: weights/bias on the ACT HWDGE ring, emb/state chunk-sliced on the SP ring,
    e3..e0 via SWDGE; stores ride SP after all loads (HWDGE rings are FIFO - a store
    that waits on compute must never sit ahead of a load).

Self-contained: hardcodes shapes from the problem spec (B=16384, H=256, ch=16, G=8).
"""
import contextlib
import os
import sys

import numpy as np

for _p in ('/opt/trn_rl_repo', '/root/.axon_site/_ro/trn_rl_repo'):
    if _p not in sys.path and os.path.isdir(_p):
        sys.path.insert(0, _p)

import ml_dtypes  # noqa: E402

import concourse.bacc as bacc  # noqa: E402
import concourse.bass as bass  # noqa: E402
import concourse.mybir as mybir  # noqa: E402
import concourse.tile as tile  # noqa: E402
from concourse.bass_utils import run_bass_kernel_spmd  # noqa: E402

BF16 = mybir.dt.bfloat16
F32 = mybir.dt.float32
AF = mybir.ActivationFunctionType
ALU = mybir.AluOpType

B = 16384
NCORES = 8
BC = B // NCORES          # 2048 batch rows per core
NCHUNK = 4
NB = BC // NCHUNK         # 512 batch columns per chunk (one PSUM bank per f32 tile)
CH = 16

LAST_RESULTS = None       # test.py reads profiling info from here

# ---------------------------------------------------------------------------
# Host-side weight preprocessing
# ---------------------------------------------------------------------------


def _sep_mats(dw, pw, g, b, F_in, mode):
    """Combined depthwise+pointwise+BN-gain matrix (F_in*ch, F_out*ch) + bias vec."""
    dw = np.asarray(dw, np.float32)
    pw = np.asarray(pw, np.float32)
    g = np.asarray(g, np.float32)
    b = np.asarray(b, np.float32)
    kf = dw.shape[1]
    ch = dw.shape[3]
    dwv = dw[0, :, 0, :]
    pwm = pw[0, 0]
    if mode == 'same':
        F_out = F_in
        M = np.zeros((F_in * ch, F_out * ch), np.float32)
        off = 1 if kf == 3 else 0
        for fo in range(F_out):
            for t in range(kf):
                fi = fo + t - off
                if 0 <= fi < F_in:
                    M[fi * ch:(fi + 1) * ch, fo * ch:(fo + 1) * ch] += dwv[t][:, None] * pwm
    else:  # 'tr': lhs_dilation 2, pad (1,2), kf=3
        F_out = 2 * F_in
        M = np.zeros((F_in * ch, F_out * ch), np.float32)
        for fo in range(F_out):
            for t in range(3):
                j = fo + t - 1
                if j >= 0 and j % 2 == 0 and j // 2 < F_in:
                    fi = j // 2
                    M[fi * ch:(fi + 1) * ch, fo * ch:(fo + 1) * ch] += dwv[t][:, None] * pwm
    M = M * np.tile(g, F_out)[None, :]
    return M, np.tile(b, F_out)


def _final_mat(out_w, out_g, out_b, F=32):
    out_w = np.asarray(out_w, np.float32)
    ch = out_w.shape[2]
    M = np.zeros((F * ch, F), np.float32)
    for fo in range(F):
        for t in range(3):
            fi = fo + t - 1
            if 0 <= fi < F:
                M[fi * ch:(fi + 1) * ch, fo] += out_w[0, t, :, 0]
    M = M * float(np.asarray(out_g).reshape(-1)[0])
    return M, np.full((F,), float(np.asarray(out_b).reshape(-1)[0]), np.float32)


def _grouped_bd(w):
    w = np.asarray(w, np.float32)
    gn, ig, og = w.shape
    M = np.zeros((gn * ig, gn * og), np.float32)
    for gi in range(gn):
        M[gi * ig:(gi + 1) * ig, gi * og:(gi + 1) * og] = w[gi]
    return M


def _tiles_of(M):
    """Split (K, Mo) into 128x<=128 lhsT tiles, dropping all-zero tiles."""
    K, Mo = M.shape
    out = []
    for kc in range(0, K, 128):
        for mc in range(0, Mo, 128):
            t = M[kc:kc + 128, mc:mc + 128]
            if not np.any(t):
                continue
            out.append((kc // 128, mc // 128, np.ascontiguousarray(t)))
    return out


class _WeightPack:
    def __init__(self):
        self.wcols = []
        self.woff = 0
        self.wmap = {}     # name -> list of (kc, mc, col_offset, width)
        self.bcols = []
        self.bmap = {}     # name -> list of column indices (one per 128-row tile)

    def add_w(self, name, tiles):
        lst = []
        for kc, mc, t in tiles:
            k, m = t.shape
            buf = np.zeros((128, m), np.float32)
            buf[:k] = t
            lst.append((kc, mc, self.woff, m))
            self.wcols.append(buf)
            self.woff += m
        self.wmap[name] = lst

    def add_b(self, name, vec):
        vec = np.asarray(vec, np.float32).reshape(-1)
        if not np.any(vec):
            self.bmap[name] = [None] * ((len(vec) + 127) // 128)
            return
        cols = []
        for mc in range(0, len(vec), 128):
            seg = vec[mc:mc + 128]
            buf = np.zeros((128,), np.float32)
            buf[:len(seg)] = seg
            cols.append(len(self.bcols))
            self.bcols.append(buf)
        self.bmap[name] = cols

    def finalize(self):
        wts = np.concatenate(self.wcols, axis=1).astype(ml_dtypes.bfloat16)
        if self.bcols:
            bias = np.stack(self.bcols, axis=1).astype(np.float32)
        else:
            bias = np.zeros((128, 1), np.float32)
        return wts, bias


def _prepare(params):
    p = {k: np.asarray(v, np.float32) for k, v in params.items()}
    pk = _WeightPack()

    pk.add_w('gl_in', _tiles_of(_grouped_bd(p['w_in'])))
    pk.add_w('k', _tiles_of(p['gru_k']))
    pk.add_w('rk', _tiles_of(p['gru_rk']))
    pk.add_w('gl_out', _tiles_of(_grouped_bd(p['w_out'])))

    b0, b1 = p['gru_b'][0], p['gru_b'][1]
    pk.add_b('bzr', (b0 + b1)[0:512])
    pk.add_b('b0n', b0[512:768])
    pk.add_b('b1n', b1[512:768])

    for name, fin, mode in [('c3p', 8, 'same'), ('t3', 8, 'same'),
                            ('c2p', 8, 'same'), ('t2', 8, 'tr'),
                            ('c1p', 16, 'same'), ('t1', 16, 'tr'),
                            ('c0p', 32, 'same')]:
        M, bv = _sep_mats(p[name + '_dw'], p[name + '_pw'], p[name + '_g'],
                          p[name + '_b'], fin, mode)
        pk.add_w(name, _tiles_of(M))
        pk.add_b(name, bv)

    Mf, bf = _final_mat(p['out_w'], p['out_g'], p['out_b'], 32)
    pk.add_w('fin', _tiles_of(Mf))
    pk.add_b('fin', bf)
    pk.glin_wcols = max(off + m for (_, _, off, m) in pk.wmap['gl_in'])
    pk.gru_wcols = max(off + m for nm in ('gl_in', 'k', 'rk', 'gl_out')
                       for (_, _, off, m) in pk.wmap[nm])
    return pk


# ---------------------------------------------------------------------------
# Device kernel
# ---------------------------------------------------------------------------

def _build_and_emit(pk):
    nc = bacc.Bacc('TRN2', target_bir_lowering=False, debug=False,
                   num_devices=NCORES)

    dr = {}
    for name, shape, dt, kind in [
            ('embT', [256, BC], BF16, 'ExternalInput'),
            ('stateT', [256, BC], BF16, 'ExternalInput'),
            ('e3T', [128, BC], BF16, 'ExternalInput'),
            ('e2T', [128, BC], BF16, 'ExternalInput'),
            ('e1T', [256, BC], BF16, 'ExternalInput'),
            ('e0T', [512, BC], BF16, 'ExternalInput'),
            ('wts', [128, pk.woff], BF16, 'ExternalInput'),
            ('bias', [128, max(len(pk.bcols), 1)], F32, 'ExternalInput'),
            ('mT', [32, BC], BF16, 'ExternalOutput'),
            ('newstT', [256, BC], BF16, 'ExternalOutput')]:
        dr[name] = nc.dram_tensor(name, shape, dt, kind=kind).ap()

    wmap, bmap = pk.wmap, pk.bmap

    with contextlib.ExitStack() as ctx:
        tc = ctx.enter_context(tile.TileContext(nc))
        consts = ctx.enter_context(tc.tile_pool(name='consts', bufs=1))
        inp = ctx.enter_context(tc.tile_pool(name='inp', bufs=1))
        work = ctx.enter_context(tc.tile_pool(name='work', bufs=2))
        pers = ctx.enter_context(tc.tile_pool(name='pers', bufs=NCHUNK))
        outp = ctx.enter_context(tc.tile_pool(name='outp', bufs=1))
        psum = ctx.enter_context(tc.tile_pool(name='psum', bufs=4, space='PSUM'))
        cpsum = ctx.enter_context(tc.tile_pool(name='cpsum', bufs=4, space='PSUM'))

        # ---- constants ----
        nbias = max(len(pk.bcols), 1)
        wts = consts.tile([128, pk.woff], BF16, tag='wts', name='wts_sb')
        nc.scalar.dma_start(wts[:, 0:pk.glin_wcols], dr['wts'][:, 0:pk.glin_wcols])
        nc.scalar.dma_start(wts[:, pk.glin_wcols:pk.gru_wcols],
                            dr['wts'][:, pk.glin_wcols:pk.gru_wcols])
        nc.scalar.dma_start(wts[:, pk.gru_wcols:], dr['wts'][:, pk.gru_wcols:])
        bias = consts.tile([128, nbias], F32, tag='bias', name='bias_sb')
        if pk.bcols:
            nc.scalar.dma_start(bias[:], dr['bias'][:])

        def Wap(name, idx):
            kc, mc, off, m = wmap[name][idx]
            return wts[:, off:off + m]

        def Bv(name, i):
            j = bmap[name][i]
            if j is None:
                return None
            return bias[:, j:j + 1]

        def act(out, in_, func, bname=None, bi=0, psl=None):
            bv = Bv(bname, bi) if bname is not None else None
            if bv is None:
                nc.scalar.activation(out, in_, func)
            else:
                nc.scalar.activation(out, in_, func,
                                     bias=bv if psl is None else bv[psl])

        # ---- load inputs (feature-major bf16) ----
        ins = {}
        for name, nrow in [('embT', 256), ('stateT', 256), ('e3T', 128),
                           ('e2T', 128), ('e1T', 256), ('e0T', 512)]:
            ins[name] = [inp.tile([128, BC], BF16, tag=f'{name}{t}', name=f'{name}{t}')
                         for t in range(nrow // 128)]
        for c in range(NCHUNK):
            S = slice(c * NB, (c + 1) * NB)
            for name in ('embT', 'stateT'):
                for t, tl in enumerate(ins[name]):
                    nc.sync.dma_start(tl[:, S], dr[name][t * 128:(t + 1) * 128, S])
        for name in ('e3T', 'e2T', 'e1T', 'e0T'):
            for t, tl in enumerate(ins[name]):
                nc.gpsimd.dma_start(tl[:], dr[name][t * 128:(t + 1) * 128, :])

        # ---- PE warmup: dummy matmuls during the DMA head keep the HAM
        # clock-gate at 8/8 so the real matmul stream starts warm ----
        wz = consts.tile([128, NB], BF16, tag='wz', name='wz')
        nc.vector.memset(wz[:], 0.0)
        ps_w = cpsum.tile([128, NB], F32, tag='cps', name='cps')
        for i in range(24):
            nc.tensor.matmul(ps_w[:], wz[:, 0:128], wz[:], start=i == 0, stop=i == 23)

        def keepwarm(n=4):
            # tiny dummy matmuls: keep the PE HAM activity window busy across
            # scheduling seams so the real stream stays at 2.4 GHz
            ps = cpsum.tile([128, 128], F32, tag='cps', name='cps')
            for i in range(n):
                nc.tensor.matmul(ps[:], wz[:, 0:128], wz[:, 0:128],
                                 start=i == 0, stop=i == n - 1)

        # ---- output staging ----
        st_out = [outp.tile([128, BC], BF16, tag=f'st{t}', name=f'st{t}')
                  for t in range(2)]
        m_out = outp.tile([32, BC], BF16, tag='m', name='m_out')

        def conv_mm(name, rhs_tiles, extra_rhs=None):
            tiles = wmap[name]
            by_m = {}
            for (kc, mc, off, m) in tiles:
                by_m.setdefault(mc, []).append((kc, off, m))
            pst = {}
            for mc, lst in sorted(by_m.items()):
                mwidth = lst[0][2]
                ps = cpsum.tile([mwidth, NB], F32, tag='cps', name='cps')
                seq = []
                for kc, off, m in lst:
                    seq.append((wts[:, off:off + m], rhs_tiles[kc]))
                    if extra_rhs is not None:
                        seq.append((wts[:, off:off + m], extra_rhs[kc]))
                for i, (w, rhs) in enumerate(seq):
                    nc.tensor.matmul(ps[:], w, rhs,
                                     start=i == 0, stop=i == len(seq) - 1)
                pst[mc] = ps
            return [pst[mc] for mc in sorted(pst)]

        def relu_b(name, pst, eng, pool, tagp=None):
            outs = []
            tagp = tagp or name
            for mi, ps in enumerate(pst):
                o = pool.tile([ps.shape[0], NB], BF16, tag=f'{tagp}r{mi}',
                              name=f'{tagp}r{mi}')
                bv = Bv(name, mi)
                if eng == 'act':
                    if bv is None:
                        nc.scalar.activation(o[:], ps[:], AF.Relu)
                    else:
                        nc.scalar.activation(o[:], ps[:], AF.Relu, bias=bv)
                else:
                    if bv is None:
                        nc.vector.tensor_scalar_max(o[:], ps[:], 0.0)
                    else:
                        nc.vector.tensor_scalar(o[:], ps[:], bv, 0.0,
                                                op0=ALU.add, op1=ALU.max)
                outs.append(o)
            return outs

        P = {}   # per-chunk persistent tiles

        def phase_gru(c):
            """GRU matmuls + gate chain + the four input-side conv blocks."""
            S = slice(c * NB, (c + 1) * NB)
            hT = [ins['stateT'][t][:, S] for t in range(2)]

            ps_gl = []
            for t in range(2):
                ps = psum.tile([128, NB], F32, tag='ps', name='ps')
                nc.tensor.matmul(ps[:], Wap('gl_in', t), ins['embT'][t][:, S],
                                 start=True, stop=True)
                ps_gl.append(ps)
            xin = []
            for t in range(2):
                xt = pers.tile([128, NB], BF16, tag=f'xin{t}', name=f'xin{t}', bufs=3)
                nc.scalar.activation(xt[:], ps_gl[t][:], AF.Relu)
                xin.append(xt)

            zr = []
            for mi in range(4):
                ps = psum.tile([128, NB], F32, tag='ps', name='ps')
                for kc in range(2):
                    nc.tensor.matmul(ps[:], Wap('rk', kc * 6 + mi), hT[kc],
                                     start=kc == 0, stop=False)
                for kc in range(2):
                    nc.tensor.matmul(ps[:], Wap('k', kc * 6 + mi), xin[kc][:],
                                     start=False, stop=kc == 1)
                g = work.tile([128, NB], BF16, tag=f'zr{mi}', name=f'zr{mi}')
                act(g[:], ps[:], AF.Sigmoid, 'bzr', mi)
                zr.append(g)
            z, r = zr[0:2], zr[2:4]

            zh = []
            for t in range(2):
                zht = work.tile([128, NB], BF16, tag=f'zh{t}', name=f'zh{t}')
                nc.vector.tensor_tensor(zht[:], z[t][:], hT[t], op=ALU.mult)
                zh.append(zht)

            for t in range(2):
                mi = 4 + t
                ps_hn = psum.tile([128, NB], F32, tag='ps', name='ps')
                for kc in range(2):
                    nc.tensor.matmul(ps_hn[:], Wap('rk', kc * 6 + mi), hT[kc],
                                     start=kc == 0, stop=kc == 1)
                ps_xn = psum.tile([128, NB], F32, tag='ps', name='ps')
                for kc in range(2):
                    nc.tensor.matmul(ps_xn[:], Wap('k', kc * 6 + mi), xin[kc][:],
                                     start=kc == 0, stop=kc == 1)
                hnr = work.tile([128, NB], BF16, tag=f'hnr{t}', name=f'hnr{t}')
                b1nv = Bv('b1n', t)
                if b1nv is None:
                    nc.vector.tensor_tensor(hnr[:], ps_hn[:], r[t][:], op=ALU.mult)
                else:
                    nc.vector.scalar_tensor_tensor(hnr[:], ps_hn[:], b1nv,
                                                   r[t][:], op0=ALU.add,
                                                   op1=ALU.mult)
                nin = work.tile([128, NB], BF16, tag=f'nin{t}', name=f'nin{t}')
                nc.vector.tensor_tensor(nin[:], ps_xn[:], hnr[:], op=ALU.add)
                nt = work.tile([128, NB], BF16, tag=f'n{t}', name=f'n{t}')
                act(nt[:], nin[:], AF.Tanh, 'b0n', t)
                zn = work.tile([128, NB], BF16, tag=f'zn{t}', name=f'zn{t}')
                nc.vector.tensor_tensor(zn[:], z[t][:], nt[:], op=ALU.mult)
                nmz = work.tile([128, NB], BF16, tag=f'nmz{t}', name=f'nmz{t}')
                nc.vector.tensor_tensor(nmz[:], nt[:], zn[:], op=ALU.subtract)
                hpt = st_out[t][:, S]
                nc.vector.tensor_tensor(hpt, zh[t][:], nmz[:], op=ALU.add)

            for t in range(2):
                nc.sync.dma_start(dr['newstT'][t * 128:(t + 1) * 128, S],
                                  st_out[t][:, S])

            # input-side conv blocks (only need e*T): PE filler for gate drain
            rc3 = relu_b('c3p', conv_mm('c3p', [ins['e3T'][0][:, S]]), 'vec', pers)
            rc2 = relu_b('c2p', conv_mm('c2p', [ins['e2T'][0][:, S]]), 'vec', pers)
            rc1 = relu_b('c1p', conv_mm('c1p', [t[:, S] for t in ins['e1T']]),
                         'vec', pers)
            rc0 = relu_b('c0p', conv_mm('c0p', [t[:, S] for t in ins['e0T']]),
                         'act', pers)
            P[c] = dict(xin=xin, rc3=rc3, rc2=rc2, rc1=rc1, rc0=rc0)

        def phase_glout(c):
            """gl_out (gout = hp + xin folded as extra contraction) -> emb4."""
            S = slice(c * NB, (c + 1) * NB)
            ps = cpsum.tile([128, NB], F32, tag='cps', name='cps')
            seqg = []
            for i in range(2):
                kc = wmap['gl_out'][i][0]
                seqg.append((Wap('gl_out', i), st_out[kc][:, S]))
            for i in range(2):
                kc = wmap['gl_out'][i][0]
                seqg.append((Wap('gl_out', i), P[c]['xin'][kc][:]))
            for i, (w, rhs) in enumerate(seqg):
                nc.tensor.matmul(ps[:], w, rhs, start=i == 0,
                                 stop=i == len(seqg) - 1)
            emb4 = pers.tile([128, NB], BF16, tag='emb4', name='emb4', bufs=3)
            nc.scalar.activation(emb4[:], ps[:], AF.Relu)
            P[c]['emb4'] = emb4

        def stage_t3(c):
            P[c]['x3'] = relu_b('t3', conv_mm('t3', [P[c]['rc3'][0][:]],
                                              extra_rhs=[P[c]['emb4'][:]]),
                                'act', pers)

        def stage_t2(c):
            P[c]['x2'] = relu_b('t2', conv_mm('t2', [P[c]['rc2'][0][:]],
                                              extra_rhs=[P[c]['x3'][0][:]]),
                                'act', pers)

        def stage_t1(c):
            x1 = []
            for t in range(2):
                xt = work.tile([128, NB], BF16, tag=f'x1_{t}', name=f'x1_{t}')
                nc.vector.tensor_tensor(xt[:], P[c]['rc1'][t][:], P[c]['x2'][t][:],
                                        op=ALU.add)
                x1.append(xt)
            P[c]['x1t'] = relu_b('t1', conv_mm('t1', [x[:] for x in x1]),
                                 'vec', pers)

        def stage_fin(c):
            S = slice(c * NB, (c + 1) * NB)
            psf = cpsum.tile([32, NB], F32, tag='cps', name='cps')
            seqf = []
            for (kc, mc, off, m) in wmap['fin']:
                seqf.append((wts[:, off:off + m], P[c]['rc0'][kc][:]))
                seqf.append((wts[:, off:off + m], P[c]['x1t'][kc][:]))
            for i, (w, rhs) in enumerate(seqf):
                nc.tensor.matmul(psf[:], w, rhs, start=i == 0,
                                 stop=i == len(seqf) - 1)
            act(m_out[:, S], psf[:], AF.Sigmoid, 'fin', 0, psl=slice(0, 32))
            nc.sync.dma_start(dr['mT'][:, S], m_out[:, S])

        # Hand-pipelined schedule: glout(c) is skewed two GRU chunks later so
        # the gate chain of chunk c drains while PE runs chunks c+1, c+2.
        phase_gru(0)
        keepwarm()
        phase_gru(1)
        keepwarm()
        phase_glout(0)
        phase_gru(2)
        keepwarm()
        phase_glout(1)
        phase_gru(3)
        phase_glout(2)
        stage_t3(0)
        phase_glout(3)
        stage_t3(1)
        stage_t2(0)
        stage_t3(2)
        stage_t2(1)
        stage_t3(3)
        stage_t2(2)
        keepwarm()
        stage_t1(0)
        stage_t2(3)
        stage_t1(1)
        keepwarm()
        stage_fin(0)
        stage_t1(2)
        keepwarm()
        stage_fin(1)
        stage_t1(3)
        stage_fin(2)
        stage_fin(3)

    nc.compile()
    return nc


# ---------------------------------------------------------------------------
# Public entry point
# ---------------------------------------------------------------------------

def make_in_maps(emb, e3, e2, e1, e0, erb_dec_state, wts, bias):
    bf = ml_dtypes.bfloat16
    embT = np.ascontiguousarray(np.asarray(emb, np.float32).T).astype(bf)
    stT = np.ascontiguousarray(np.asarray(erb_dec_state, np.float32).T).astype(bf)
    e3T = np.ascontiguousarray(np.asarray(e3, np.float32).reshape(B, -1).T).astype(bf)
    e2T = np.ascontiguousarray(np.asarray(e2, np.float32).reshape(B, -1).T).astype(bf)
    e1T = np.ascontiguousarray(np.asarray(e1, np.float32).reshape(B, -1).T).astype(bf)
    e0T = np.ascontiguousarray(np.asarray(e0, np.float32).reshape(B, -1).T).astype(bf)
    in_maps = []
    for i in range(NCORES):
        S = slice(i * BC, (i + 1) * BC)
        in_maps.append({
            'embT': np.ascontiguousarray(embT[:, S]),
            'stateT': np.ascontiguousarray(stT[:, S]),
            'e3T': np.ascontiguousarray(e3T[:, S]),
            'e2T': np.ascontiguousarray(e2T[:, S]),
            'e1T': np.ascontiguousarray(e1T[:, S]),
            'e0T': np.ascontiguousarray(e0T[:, S]),
            'wts': wts,
            'bias': bias,
        })
    return in_maps


def _ensure_ntff_hook():
    """Register the axon NTFF profiling hook if the image's antenv lacks it."""
    try:
        from antenv.axon_hooks import get_axon_ntff_profile_hook  # noqa: F401
        return
    except ImportError:
        pass
    import types

    import antenv
    mod = types.ModuleType('antenv.axon_hooks')
    _h = [None]
    mod.get_axon_ntff_profile_hook = lambda: _h[0]
    mod.set_axon_ntff_profile_hook = lambda hook: _h.__setitem__(0, hook)
    sys.modules['antenv.axon_hooks'] = mod
    antenv.axon_hooks = mod
    try:
        from trn_agent_boot.trn_boot import _ntff_profile_via_ctypes
        mod.set_axon_ntff_profile_hook(
            _ntff_profile_via_ctypes('/opt/axon/libaxon_pjrt.so'))
    except Exception:
        pass


def kernel(emb, e3, e2, e1, e0, erb_dec_state, params):
    global LAST_RESULTS
    if os.environ.get('BASS_TRACE'):
        _ensure_ntff_hook()
    pk = _prepare(params)
    wts, bias = pk.finalize()
    nc = _build_and_emit(pk)
    in_maps = make_in_maps(emb, e3, e2, e1, e0, erb_dec_state, wts, bias)

    res = run_bass_kernel_spmd(nc, in_maps, list(range(NCORES)),
                               trace=bool(os.environ.get('BASS_TRACE')))
    LAST_RESULTS = res

    m = np.empty((B, 32), np.float32)
    new_state = np.empty((B, 256), np.float32)
    for i in range(NCORES):
        S = slice(i * BC, (i + 1) * BC)
        m[S] = np.asarray(res.results[i]['mT']).astype(np.float32).T
        new_state[S] = np.asarray(res.results[i]['newstT']).astype(np.float32).T
    return m.reshape(B, 1, 1, 32), new_state


# revision 35
# speedup vs baseline: 1.0204x; 1.0058x over previous
"""Trainium2 Bass kernel for nn_ErbDecoderStep (GRU + grouped-linear + separable-conv decoder).

Strategy:
  - Pure data parallel: batch B=16384 sharded as 2048 rows per NeuronCore (8 cores).
  - All device compute is feature-major: activations live in SBUF as (features, batch)
    tiles so every matmul contracts over the partition axis. The host pre-transposes
    (and casts to bf16) the inputs and transposes the two outputs back.
  - Depthwise+pointwise+folded-BN of every separable conv block is combined on the host
    into one small banded matrix per block (transpose convs become banded upsampling
    matrices); residual adds are folded into the following matmul as an extra
    contraction block reusing the same weight tiles, or done on DVE.
  - bf16 matmuls with fp32 PSUM accumulation; bf16 DRAM outputs upcast on host.
  - Hand-pipelined schedule: per-chunk GRU groups (PSUM bank released per gate via
    [rk,rk,k,k]+sigmoid), gl_out skewed two chunks later so the gate chain drains off
    the PE critical path, then stage-major conv rounds (t3/t2/t1/fin across chunks).
  - DMA: weights/bias on the ACT HWDGE ring, emb/state chunk-sliced on the SP ring,
    e3..e0 via SWDGE; stores ride SP after all loads (HWDGE rings are FIFO - a store
    that waits on compute must never sit ahead of a load).

Self-contained: hardcodes shapes from the problem spec (B=16384, H=256, ch=16, G=8).
"""
import contextlib
import os
import sys

import numpy as np

for _p in ('/opt/trn_rl_repo', '/root/.axon_site/_ro/trn_rl_repo'):
    if _p not in sys.path and os.path.isdir(_p):
        sys.path.insert(0, _p)

import ml_dtypes  # noqa: E402

import concourse.bacc as bacc  # noqa: E402
import concourse.bass as bass  # noqa: E402
import concourse.mybir as mybir  # noqa: E402
import concourse.tile as tile  # noqa: E402
from concourse.bass_utils import run_bass_kernel_spmd  # noqa: E402

BF16 = mybir.dt.bfloat16
F32 = mybir.dt.float32
AF = mybir.ActivationFunctionType
ALU = mybir.AluOpType

B = 16384
NCORES = 8
BC = B // NCORES          # 2048 batch rows per core
NCHUNK = 4
NB = BC // NCHUNK         # 512 batch columns per chunk (one PSUM bank per f32 tile)
CH = 16

LAST_RESULTS = None       # test.py reads profiling info from here

# ---------------------------------------------------------------------------
# Host-side weight preprocessing
# ---------------------------------------------------------------------------


def _sep_mats(dw, pw, g, b, F_in, mode):
    """Combined depthwise+pointwise+BN-gain matrix (F_in*ch, F_out*ch) + bias vec."""
    dw = np.asarray(dw, np.float32)
    pw = np.asarray(pw, np.float32)
    g = np.asarray(g, np.float32)
    b = np.asarray(b, np.float32)
    kf = dw.shape[1]
    ch = dw.shape[3]
    dwv = dw[0, :, 0, :]
    pwm = pw[0, 0]
    if mode == 'same':
        F_out = F_in
        M = np.zeros((F_in * ch, F_out * ch), np.float32)
        off = 1 if kf == 3 else 0
        for fo in range(F_out):
            for t in range(kf):
                fi = fo + t - off
                if 0 <= fi < F_in:
                    M[fi * ch:(fi + 1) * ch, fo * ch:(fo + 1) * ch] += dwv[t][:, None] * pwm
    else:  # 'tr': lhs_dilation 2, pad (1,2), kf=3
        F_out = 2 * F_in
        M = np.zeros((F_in * ch, F_out * ch), np.float32)
        for fo in range(F_out):
            for t in range(3):
                j = fo + t - 1
                if j >= 0 and j % 2 == 0 and j // 2 < F_in:
                    fi = j // 2
                    M[fi * ch:(fi + 1) * ch, fo * ch:(fo + 1) * ch] += dwv[t][:, None] * pwm
    M = M * np.tile(g, F_out)[None, :]
    return M, np.tile(b, F_out)


def _final_mat(out_w, out_g, out_b, F=32):
    out_w = np.asarray(out_w, np.float32)
    ch = out_w.shape[2]
    M = np.zeros((F * ch, F), np.float32)
    for fo in range(F):
        for t in range(3):
            fi = fo + t - 1
            if 0 <= fi < F:
                M[fi * ch:(fi + 1) * ch, fo] += out_w[0, t, :, 0]
    M = M * float(np.asarray(out_g).reshape(-1)[0])
    return M, np.full((F,), float(np.asarray(out_b).reshape(-1)[0]), np.float32)


def _grouped_bd(w):
    w = np.asarray(w, np.float32)
    gn, ig, og = w.shape
    M = np.zeros((gn * ig, gn * og), np.float32)
    for gi in range(gn):
        M[gi * ig:(gi + 1) * ig, gi * og:(gi + 1) * og] = w[gi]
    return M


def _tiles_of(M):
    """Split (K, Mo) into 128x<=128 lhsT tiles, dropping all-zero tiles."""
    K, Mo = M.shape
    out = []
    for kc in range(0, K, 128):
        for mc in range(0, Mo, 128):
            t = M[kc:kc + 128, mc:mc + 128]
            if not np.any(t):
                continue
            out.append((kc // 128, mc // 128, np.ascontiguousarray(t)))
    return out


class _WeightPack:
    def __init__(self):
        self.wcols = []
        self.woff = 0
        self.wmap = {}     # name -> list of (kc, mc, col_offset, width)
        self.bcols = []
        self.bmap = {}     # name -> list of column indices (one per 128-row tile)

    def add_w(self, name, tiles):
        lst = []
        for kc, mc, t in tiles:
            k, m = t.shape
            buf = np.zeros((128, m), np.float32)
            buf[:k] = t
            lst.append((kc, mc, self.woff, m))
            self.wcols.append(buf)
            self.woff += m
        self.wmap[name] = lst

    def add_b(self, name, vec):
        vec = np.asarray(vec, np.float32).reshape(-1)
        if not np.any(vec):
            self.bmap[name] = [None] * ((len(vec) + 127) // 128)
            return
        cols = []
        for mc in range(0, len(vec), 128):
            seg = vec[mc:mc + 128]
            buf = np.zeros((128,), np.float32)
            buf[:len(seg)] = seg
            cols.append(len(self.bcols))
            self.bcols.append(buf)
        self.bmap[name] = cols

    def finalize(self):
        wts = np.concatenate(self.wcols, axis=1).astype(ml_dtypes.bfloat16)
        if self.bcols:
            bias = np.stack(self.bcols, axis=1).astype(np.float32)
        else:
            bias = np.zeros((128, 1), np.float32)
        return wts, bias


def _prepare(params):
    p = {k: np.asarray(v, np.float32) for k, v in params.items()}
    pk = _WeightPack()

    pk.add_w('gl_in', _tiles_of(_grouped_bd(p['w_in'])))
    pk.add_w('k', _tiles_of(p['gru_k']))
    pk.add_w('rk', _tiles_of(p['gru_rk']))
    pk.add_w('gl_out', _tiles_of(_grouped_bd(p['w_out'])))

    b0, b1 = p['gru_b'][0], p['gru_b'][1]
    pk.add_b('bzr', (b0 + b1)[0:512])
    pk.add_b('b0n', b0[512:768])
    pk.add_b('b1n', b1[512:768])

    for name, fin, mode in [('c3p', 8, 'same'), ('t3', 8, 'same'),
                            ('c2p', 8, 'same'), ('t2', 8, 'tr'),
                            ('c1p', 16, 'same'), ('t1', 16, 'tr'),
                            ('c0p', 32, 'same')]:
        M, bv = _sep_mats(p[name + '_dw'], p[name + '_pw'], p[name + '_g'],
                          p[name + '_b'], fin, mode)
        pk.add_w(name, _tiles_of(M))
        pk.add_b(name, bv)

    Mf, bf = _final_mat(p['out_w'], p['out_g'], p['out_b'], 32)
    pk.add_w('fin', _tiles_of(Mf))
    pk.add_b('fin', bf)
    pk.glin_wcols = max(off + m for (_, _, off, m) in pk.wmap['gl_in'])
    pk.gru_wcols = max(off + m for nm in ('gl_in', 'k', 'rk', 'gl_out')
                       for (_, _, off, m) in pk.wmap[nm])
    return pk


# ---------------------------------------------------------------------------
# Device kernel
# ---------------------------------------------------------------------------

def _build_and_emit(pk):
    nc = bacc.Bacc('TRN2', target_bir_lowering=False, debug=False,
                   num_devices=NCORES)

    dr = {}
    for name, shape, dt, kind in [
            ('embT', [256, BC], BF16, 'ExternalInput'),
            ('stateT', [256, BC], BF16, 'ExternalInput'),
            ('e3T', [128, BC], BF16, 'ExternalInput'),
            ('e2T', [128, BC], BF16, 'ExternalInput'),
            ('e1T', [256, BC], BF16, 'ExternalInput'),
            ('e0T', [512, BC], BF16, 'ExternalInput'),
            ('wts', [128, pk.woff], BF16, 'ExternalInput'),
            ('bias', [128, max(len(pk.bcols), 1)], F32, 'ExternalInput'),
            ('mT', [32, BC], BF16, 'ExternalOutput'),
            ('newstT', [256, BC], BF16, 'ExternalOutput')]:
        dr[name] = nc.dram_tensor(name, shape, dt, kind=kind).ap()

    wmap, bmap = pk.wmap, pk.bmap

    with contextlib.ExitStack() as ctx:
        tc = ctx.enter_context(tile.TileContext(nc))
        consts = ctx.enter_context(tc.tile_pool(name='consts', bufs=1))
        inp = ctx.enter_context(tc.tile_pool(name='inp', bufs=1))
        work = ctx.enter_context(tc.tile_pool(name='work', bufs=2))
        pers = ctx.enter_context(tc.tile_pool(name='pers', bufs=NCHUNK))
        outp = ctx.enter_context(tc.tile_pool(name='outp', bufs=1))
        psum = ctx.enter_context(tc.tile_pool(name='psum', bufs=4, space='PSUM'))
        cpsum = ctx.enter_context(tc.tile_pool(name='cpsum', bufs=4, space='PSUM'))

        # ---- constants ----
        nbias = max(len(pk.bcols), 1)
        wts = consts.tile([128, pk.woff], BF16, tag='wts', name='wts_sb')
        nc.scalar.dma_start(wts[:, 0:pk.glin_wcols], dr['wts'][:, 0:pk.glin_wcols])
        nc.scalar.dma_start(wts[:, pk.glin_wcols:pk.gru_wcols],
                            dr['wts'][:, pk.glin_wcols:pk.gru_wcols])
        nc.scalar.dma_start(wts[:, pk.gru_wcols:], dr['wts'][:, pk.gru_wcols:])
        bias = consts.tile([128, nbias], F32, tag='bias', name='bias_sb')
        if pk.bcols:
            nc.scalar.dma_start(bias[:], dr['bias'][:])

        def Wap(name, idx):
            kc, mc, off, m = wmap[name][idx]
            return wts[:, off:off + m]

        def Bv(name, i):
            j = bmap[name][i]
            if j is None:
                return None
            return bias[:, j:j + 1]

        def act(out, in_, func, bname=None, bi=0, psl=None):
            bv = Bv(bname, bi) if bname is not None else None
            if bv is None:
                nc.scalar.activation(out, in_, func)
            else:
                nc.scalar.activation(out, in_, func,
                                     bias=bv if psl is None else bv[psl])

        # ---- load inputs (feature-major bf16) ----
        ins = {}
        for name, nrow in [('embT', 256), ('stateT', 256), ('e3T', 128),
                           ('e2T', 128), ('e1T', 256), ('e0T', 512)]:
            ins[name] = [inp.tile([128, BC], BF16, tag=f'{name}{t}', name=f'{name}{t}')
                         for t in range(nrow // 128)]
        for c in range(NCHUNK):
            S = slice(c * NB, (c + 1) * NB)
            for name in ('embT', 'stateT'):
                for t, tl in enumerate(ins[name]):
                    nc.sync.dma_start(tl[:, S], dr[name][t * 128:(t + 1) * 128, S])
        for name in ('e3T', 'e2T', 'e1T', 'e0T'):
            for t, tl in enumerate(ins[name]):
                nc.gpsimd.dma_start(tl[:], dr[name][t * 128:(t + 1) * 128, :])

        # ---- PE warmup: dummy matmuls during the DMA head keep the HAM
        # clock-gate at 8/8 so the real matmul stream starts warm ----
        wz = consts.tile([128, NB], BF16, tag='wz', name='wz')
        nc.vector.memset(wz[:], 0.0)
        ps_w = cpsum.tile([128, NB], F32, tag='cps', name='cps')
        for i in range(24):
            nc.tensor.matmul(ps_w[:], wz[:, 0:128], wz[:], start=i == 0, stop=i == 23)

        # ---- output staging ----
        st_out = [outp.tile([128, BC], BF16, tag=f'st{t}', name=f'st{t}')
                  for t in range(2)]
        m_out = outp.tile([32, BC], BF16, tag='m', name='m_out')

        def conv_mm(name, rhs_tiles, extra_rhs=None):
            tiles = wmap[name]
            by_m = {}
            for (kc, mc, off, m) in tiles:
                by_m.setdefault(mc, []).append((kc, off, m))
            pst = {}
            for mc, lst in sorted(by_m.items()):
                mwidth = lst[0][2]
                ps = cpsum.tile([mwidth, NB], F32, tag='cps', name='cps')
                seq = []
                for kc, off, m in lst:
                    seq.append((wts[:, off:off + m], rhs_tiles[kc]))
                    if extra_rhs is not None:
                        seq.append((wts[:, off:off + m], extra_rhs[kc]))
                for i, (w, rhs) in enumerate(seq):
                    nc.tensor.matmul(ps[:], w, rhs,
                                     start=i == 0, stop=i == len(seq) - 1)
                pst[mc] = ps
            return [pst[mc] for mc in sorted(pst)]

        def relu_b(name, pst, eng, pool, tagp=None):
            outs = []
            tagp = tagp or name
            for mi, ps in enumerate(pst):
                o = pool.tile([ps.shape[0], NB], BF16, tag=f'{tagp}r{mi}',
                              name=f'{tagp}r{mi}')
                bv = Bv(name, mi)
                if eng == 'act':
                    if bv is None:
                        nc.scalar.activation(o[:], ps[:], AF.Relu)
                    else:
                        nc.scalar.activation(o[:], ps[:], AF.Relu, bias=bv)
                else:
                    if bv is None:
                        nc.vector.tensor_scalar_max(o[:], ps[:], 0.0)
                    else:
                        nc.vector.tensor_scalar(o[:], ps[:], bv, 0.0,
                                                op0=ALU.add, op1=ALU.max)
                outs.append(o)
            return outs

        P = {}   # per-chunk persistent tiles

        def phase_gru(c):
            """GRU matmuls + gate chain + the four input-side conv blocks."""
            S = slice(c * NB, (c + 1) * NB)
            hT = [ins['stateT'][t][:, S] for t in range(2)]

            ps_gl = []
            for t in range(2):
                ps = psum.tile([128, NB], F32, tag='ps', name='ps')
                nc.tensor.matmul(ps[:], Wap('gl_in', t), ins['embT'][t][:, S],
                                 start=True, stop=True)
                ps_gl.append(ps)
            xin = []
            for t in range(2):
                xt = pers.tile([128, NB], BF16, tag=f'xin{t}', name=f'xin{t}', bufs=3)
                nc.scalar.activation(xt[:], ps_gl[t][:], AF.Relu)
                xin.append(xt)

            zr = []
            for mi in range(4):
                ps = psum.tile([128, NB], F32, tag='ps', name='ps')
                for kc in range(2):
                    nc.tensor.matmul(ps[:], Wap('rk', kc * 6 + mi), hT[kc],
                                     start=kc == 0, stop=False)
                for kc in range(2):
                    nc.tensor.matmul(ps[:], Wap('k', kc * 6 + mi), xin[kc][:],
                                     start=False, stop=kc == 1)
                g = work.tile([128, NB], BF16, tag=f'zr{mi}', name=f'zr{mi}')
                act(g[:], ps[:], AF.Sigmoid, 'bzr', mi)
                zr.append(g)
            z, r = zr[0:2], zr[2:4]

            zh = []
            for t in range(2):
                zht = work.tile([128, NB], BF16, tag=f'zh{t}', name=f'zh{t}')
                nc.vector.tensor_tensor(zht[:], z[t][:], hT[t], op=ALU.mult)
                zh.append(zht)

            for t in range(2):
                mi = 4 + t
                ps_hn = psum.tile([128, NB], F32, tag='ps', name='ps')
                for kc in range(2):
                    nc.tensor.matmul(ps_hn[:], Wap('rk', kc * 6 + mi), hT[kc],
                                     start=kc == 0, stop=kc == 1)
                ps_xn = psum.tile([128, NB], F32, tag='ps', name='ps')
                for kc in range(2):
                    nc.tensor.matmul(ps_xn[:], Wap('k', kc * 6 + mi), xin[kc][:],
                                     start=kc == 0, stop=kc == 1)
                hnr = work.tile([128, NB], BF16, tag=f'hnr{t}', name=f'hnr{t}')
                b1nv = Bv('b1n', t)
                if b1nv is None:
                    nc.vector.tensor_tensor(hnr[:], ps_hn[:], r[t][:], op=ALU.mult)
                else:
                    nc.vector.scalar_tensor_tensor(hnr[:], ps_hn[:], b1nv,
                                                   r[t][:], op0=ALU.add,
                                                   op1=ALU.mult)
                nin = work.tile([128, NB], BF16, tag=f'nin{t}', name=f'nin{t}')
                nc.vector.tensor_tensor(nin[:], ps_xn[:], hnr[:], op=ALU.add)
                nt = work.tile([128, NB], BF16, tag=f'n{t}', name=f'n{t}')
                act(nt[:], nin[:], AF.Tanh, 'b0n', t)
                zn = work.tile([128, NB], BF16, tag=f'zn{t}', name=f'zn{t}')
                nc.vector.tensor_tensor(zn[:], z[t][:], nt[:], op=ALU.mult)
                nmz = work.tile([128, NB], BF16, tag=f'nmz{t}', name=f'nmz{t}')
                nc.vector.tensor_tensor(nmz[:], nt[:], zn[:], op=ALU.subtract)
                hpt = st_out[t][:, S]
                nc.vector.tensor_tensor(hpt, zh[t][:], nmz[:], op=ALU.add)

            for t in range(2):
                nc.sync.dma_start(dr['newstT'][t * 128:(t + 1) * 128, S],
                                  st_out[t][:, S])

            # input-side conv blocks (only need e*T): PE filler for gate drain
            rc3 = relu_b('c3p', conv_mm('c3p', [ins['e3T'][0][:, S]]), 'vec', pers)
            rc2 = relu_b('c2p', conv_mm('c2p', [ins['e2T'][0][:, S]]), 'vec', pers)
            rc1 = relu_b('c1p', conv_mm('c1p', [t[:, S] for t in ins['e1T']]),
                         'vec', pers)
            rc0 = relu_b('c0p', conv_mm('c0p', [t[:, S] for t in ins['e0T']]),
                         'act', pers)
            P[c] = dict(xin=xin, rc3=rc3, rc2=rc2, rc1=rc1, rc0=rc0)

        def phase_glout(c):
            """gl_out (gout = hp + xin folded as extra contraction) -> emb4."""
            S = slice(c * NB, (c + 1) * NB)
            ps = cpsum.tile([128, NB], F32, tag='cps', name='cps')
            seqg = []
            for i in range(2):
                kc = wmap['gl_out'][i][0]
                seqg.append((Wap('gl_out', i), st_out[kc][:, S]))
            for i in range(2):
                kc = wmap['gl_out'][i][0]
                seqg.append((Wap('gl_out', i), P[c]['xin'][kc][:]))
            for i, (w, rhs) in enumerate(seqg):
                nc.tensor.matmul(ps[:], w, rhs, start=i == 0,
                                 stop=i == len(seqg) - 1)
            emb4 = pers.tile([128, NB], BF16, tag='emb4', name='emb4', bufs=3)
            nc.scalar.activation(emb4[:], ps[:], AF.Relu)
            P[c]['emb4'] = emb4

        def stage_t3(c):
            P[c]['x3'] = relu_b('t3', conv_mm('t3', [P[c]['rc3'][0][:]],
                                              extra_rhs=[P[c]['emb4'][:]]),
                                'act', pers)

        def stage_t2(c):
            P[c]['x2'] = relu_b('t2', conv_mm('t2', [P[c]['rc2'][0][:]],
                                              extra_rhs=[P[c]['x3'][0][:]]),
                                'act', pers)

        def stage_t1(c):
            x1 = []
            for t in range(2):
                xt = work.tile([128, NB], BF16, tag=f'x1_{t}', name=f'x1_{t}')
                nc.vector.tensor_tensor(xt[:], P[c]['rc1'][t][:], P[c]['x2'][t][:],
                                        op=ALU.add)
                x1.append(xt)
            P[c]['x1t'] = relu_b('t1', conv_mm('t1', [x[:] for x in x1]),
                                 'vec', pers)

        def stage_fin(c):
            S = slice(c * NB, (c + 1) * NB)
            psf = cpsum.tile([32, NB], F32, tag='cps', name='cps')
            seqf = []
            for (kc, mc, off, m) in wmap['fin']:
                seqf.append((wts[:, off:off + m], P[c]['rc0'][kc][:]))
                seqf.append((wts[:, off:off + m], P[c]['x1t'][kc][:]))
            for i, (w, rhs) in enumerate(seqf):
                nc.tensor.matmul(psf[:], w, rhs, start=i == 0,
                                 stop=i == len(seqf) - 1)
            act(m_out[:, S], psf[:], AF.Sigmoid, 'fin', 0, psl=slice(0, 32))
            nc.sync.dma_start(dr['mT'][:, S], m_out[:, S])

        # Hand-pipelined schedule: glout(c) is skewed two GRU chunks later so
        # the gate chain of chunk c drains while PE runs chunks c+1, c+2.
        phase_gru(0)
        phase_gru(1)
        phase_glout(0)
        phase_gru(2)
        phase_glout(1)
        phase_gru(3)
        phase_glout(2)
        stage_t3(0)
        phase_glout(3)
        stage_t3(1)
        stage_t2(0)
        stage_t3(2)
        stage_t2(1)
        stage_t3(3)
        stage_t2(2)
        stage_t1(0)
        stage_t2(3)
        stage_t1(1)
        stage_fin(0)
        stage_t1(2)
        stage_fin(1)
        stage_t1(3)
        stage_fin(2)
        stage_fin(3)

    nc.compile()
    return nc


# ---------------------------------------------------------------------------
# Public entry point
# ---------------------------------------------------------------------------

def make_in_maps(emb, e3, e2, e1, e0, erb_dec_state, wts, bias):
    bf = ml_dtypes.bfloat16
    embT = np.ascontiguousarray(np.asarray(emb, np.float32).T).astype(bf)
    stT = np.ascontiguousarray(np.asarray(erb_dec_state, np.float32).T).astype(bf)
    e3T = np.ascontiguousarray(np.asarray(e3, np.float32).reshape(B, -1).T).astype(bf)
    e2T = np.ascontiguousarray(np.asarray(e2, np.float32).reshape(B, -1).T).astype(bf)
    e1T = np.ascontiguousarray(np.asarray(e1, np.float32).reshape(B, -1).T).astype(bf)
    e0T = np.ascontiguousarray(np.asarray(e0, np.float32).reshape(B, -1).T).astype(bf)
    in_maps = []
    for i in range(NCORES):
        S = slice(i * BC, (i + 1) * BC)
        in_maps.append({
            'embT': np.ascontiguousarray(embT[:, S]),
            'stateT': np.ascontiguousarray(stT[:, S]),
            'e3T': np.ascontiguousarray(e3T[:, S]),
            'e2T': np.ascontiguousarray(e2T[:, S]),
            'e1T': np.ascontiguousarray(e1T[:, S]),
            'e0T': np.ascontiguousarray(e0T[:, S]),
            'wts': wts,
            'bias': bias,
        })
    return in_maps


def _ensure_ntff_hook():
    """Register the axon NTFF profiling hook if the image's antenv lacks it."""
    try:
        from antenv.axon_hooks import get_axon_ntff_profile_hook  # noqa: F401
        return
    except ImportError:
        pass
    import types

    import antenv
    mod = types.ModuleType('antenv.axon_hooks')
    _h = [None]
    mod.get_axon_ntff_profile_hook = lambda: _h[0]
    mod.set_axon_ntff_profile_hook = lambda hook: _h.__setitem__(0, hook)
    sys.modules['antenv.axon_hooks'] = mod
    antenv.axon_hooks = mod
    try:
        from trn_agent_boot.trn_boot import _ntff_profile_via_ctypes
        mod.set_axon_ntff_profile_hook(
            _ntff_profile_via_ctypes('/opt/axon/libaxon_pjrt.so'))
    except Exception:
        pass


def kernel(emb, e3, e2, e1, e0, erb_dec_state, params):
    global LAST_RESULTS
    if os.environ.get('BASS_TRACE'):
        _ensure_ntff_hook()
    pk = _prepare(params)
    wts, bias = pk.finalize()
    nc = _build_and_emit(pk)
    in_maps = make_in_maps(emb, e3, e2, e1, e0, erb_dec_state, wts, bias)

    res = run_bass_kernel_spmd(nc, in_maps, list(range(NCORES)),
                               trace=bool(os.environ.get('BASS_TRACE')))
    LAST_RESULTS = res

    m = np.empty((B, 32), np.float32)
    new_state = np.empty((B, 256), np.float32)
    for i in range(NCORES):
        S = slice(i * BC, (i + 1) * BC)
        m[S] = np.asarray(res.results[i]['mT']).astype(np.float32).T
        new_state[S] = np.asarray(res.results[i]['newstT']).astype(np.float32).T
    return m.reshape(B, 1, 1, 32), new_state


# revision 36
# speedup vs baseline: 1.0250x; 1.0045x over previous
"""Trainium2 Bass kernel for nn_ErbDecoderStep (GRU + grouped-linear + separable-conv decoder).

Strategy:
  - Pure data parallel: batch B=16384 sharded as 2048 rows per NeuronCore (8 cores).
  - All device compute is feature-major: activations live in SBUF as (features, batch)
    tiles so every matmul contracts over the partition axis. The host pre-transposes
    (and casts to bf16) the inputs and transposes the two outputs back.
  - Depthwise+pointwise+folded-BN of every separable conv block is combined on the host
    into one small banded matrix per block (transpose convs become banded upsampling
    matrices); residual adds are folded into the following matmul as an extra
    contraction block reusing the same weight tiles, or done on DVE.
  - bf16 matmuls with fp32 PSUM accumulation; bf16 DRAM outputs upcast on host.
  - Hand-pipelined schedule: per-chunk GRU groups (PSUM bank released per gate via
    [rk,rk,k,k]+sigmoid), gl_out skewed two chunks later so the gate chain drains off
    the PE critical path, then stage-major conv rounds (t3/t2/t1/fin across chunks).
  - DMA: weights/bias on the ACT HWDGE ring, emb/state chunk-sliced on the SP ring,
    e3..e0 via SWDGE; stores ride SP after all loads (HWDGE rings are FIFO - a store
    that waits on compute must never sit ahead of a load).

Self-contained: hardcodes shapes from the problem spec (B=16384, H=256, ch=16, G=8).
"""
import contextlib
import os
import sys

import numpy as np

for _p in ('/opt/trn_rl_repo', '/root/.axon_site/_ro/trn_rl_repo'):
    if _p not in sys.path and os.path.isdir(_p):
        sys.path.insert(0, _p)

import ml_dtypes  # noqa: E402

import concourse.bacc as bacc  # noqa: E402
import concourse.bass as bass  # noqa: E402
import concourse.mybir as mybir  # noqa: E402
import concourse.tile as tile  # noqa: E402
from concourse.bass_utils import run_bass_kernel_spmd  # noqa: E402

BF16 = mybir.dt.bfloat16
F32 = mybir.dt.float32
AF = mybir.ActivationFunctionType
ALU = mybir.AluOpType

B = 16384
NCORES = 8
BC = B // NCORES          # 2048 batch rows per core
NCHUNK = 4
NB = BC // NCHUNK         # 512 batch columns per chunk (one PSUM bank per f32 tile)
CH = 16

LAST_RESULTS = None       # test.py reads profiling info from here

# ---------------------------------------------------------------------------
# Host-side weight preprocessing
# ---------------------------------------------------------------------------


def _sep_mats(dw, pw, g, b, F_in, mode):
    """Combined depthwise+pointwise+BN-gain matrix (F_in*ch, F_out*ch) + bias vec."""
    dw = np.asarray(dw, np.float32)
    pw = np.asarray(pw, np.float32)
    g = np.asarray(g, np.float32)
    b = np.asarray(b, np.float32)
    kf = dw.shape[1]
    ch = dw.shape[3]
    dwv = dw[0, :, 0, :]
    pwm = pw[0, 0]
    if mode == 'same':
        F_out = F_in
        M = np.zeros((F_in * ch, F_out * ch), np.float32)
        off = 1 if kf == 3 else 0
        for fo in range(F_out):
            for t in range(kf):
                fi = fo + t - off
                if 0 <= fi < F_in:
                    M[fi * ch:(fi + 1) * ch, fo * ch:(fo + 1) * ch] += dwv[t][:, None] * pwm
    else:  # 'tr': lhs_dilation 2, pad (1,2), kf=3
        F_out = 2 * F_in
        M = np.zeros((F_in * ch, F_out * ch), np.float32)
        for fo in range(F_out):
            for t in range(3):
                j = fo + t - 1
                if j >= 0 and j % 2 == 0 and j // 2 < F_in:
                    fi = j // 2
                    M[fi * ch:(fi + 1) * ch, fo * ch:(fo + 1) * ch] += dwv[t][:, None] * pwm
    M = M * np.tile(g, F_out)[None, :]
    return M, np.tile(b, F_out)


def _final_mat(out_w, out_g, out_b, F=32):
    out_w = np.asarray(out_w, np.float32)
    ch = out_w.shape[2]
    M = np.zeros((F * ch, F), np.float32)
    for fo in range(F):
        for t in range(3):
            fi = fo + t - 1
            if 0 <= fi < F:
                M[fi * ch:(fi + 1) * ch, fo] += out_w[0, t, :, 0]
    M = M * float(np.asarray(out_g).reshape(-1)[0])
    return M, np.full((F,), float(np.asarray(out_b).reshape(-1)[0]), np.float32)


def _grouped_bd(w):
    w = np.asarray(w, np.float32)
    gn, ig, og = w.shape
    M = np.zeros((gn * ig, gn * og), np.float32)
    for gi in range(gn):
        M[gi * ig:(gi + 1) * ig, gi * og:(gi + 1) * og] = w[gi]
    return M


def _tiles_of(M):
    """Split (K, Mo) into 128x<=128 lhsT tiles, dropping all-zero tiles."""
    K, Mo = M.shape
    out = []
    for kc in range(0, K, 128):
        for mc in range(0, Mo, 128):
            t = M[kc:kc + 128, mc:mc + 128]
            if not np.any(t):
                continue
            out.append((kc // 128, mc // 128, np.ascontiguousarray(t)))
    return out


class _WeightPack:
    def __init__(self):
        self.wcols = []
        self.woff = 0
        self.wmap = {}     # name -> list of (kc, mc, col_offset, width)
        self.bcols = []
        self.bmap = {}     # name -> list of column indices (one per 128-row tile)

    def add_w(self, name, tiles):
        lst = []
        for kc, mc, t in tiles:
            k, m = t.shape
            buf = np.zeros((128, m), np.float32)
            buf[:k] = t
            lst.append((kc, mc, self.woff, m))
            self.wcols.append(buf)
            self.woff += m
        self.wmap[name] = lst

    def add_b(self, name, vec):
        vec = np.asarray(vec, np.float32).reshape(-1)
        if not np.any(vec):
            self.bmap[name] = [None] * ((len(vec) + 127) // 128)
            return
        cols = []
        for mc in range(0, len(vec), 128):
            seg = vec[mc:mc + 128]
            buf = np.zeros((128,), np.float32)
            buf[:len(seg)] = seg
            cols.append(len(self.bcols))
            self.bcols.append(buf)
        self.bmap[name] = cols

    def finalize(self):
        wts = np.concatenate(self.wcols, axis=1).astype(ml_dtypes.bfloat16)
        if self.bcols:
            bias = np.stack(self.bcols, axis=1).astype(np.float32)
        else:
            bias = np.zeros((128, 1), np.float32)
        return wts, bias


def _prepare(params):
    p = {k: np.asarray(v, np.float32) for k, v in params.items()}
    pk = _WeightPack()

    pk.add_w('gl_in', _tiles_of(_grouped_bd(p['w_in'])))
    pk.add_w('k', _tiles_of(p['gru_k']))
    pk.add_w('rk', _tiles_of(p['gru_rk']))
    pk.add_w('gl_out', _tiles_of(_grouped_bd(p['w_out'])))

    b0, b1 = p['gru_b'][0], p['gru_b'][1]
    pk.add_b('bzr', (b0 + b1)[0:512])
    pk.add_b('b0n', b0[512:768])
    pk.add_b('b1n', b1[512:768])

    for name, fin, mode in [('c3p', 8, 'same'), ('t3', 8, 'same'),
                            ('c2p', 8, 'same'), ('t2', 8, 'tr'),
                            ('c1p', 16, 'same'), ('t1', 16, 'tr'),
                            ('c0p', 32, 'same')]:
        M, bv = _sep_mats(p[name + '_dw'], p[name + '_pw'], p[name + '_g'],
                          p[name + '_b'], fin, mode)
        pk.add_w(name, _tiles_of(M))
        pk.add_b(name, bv)

    Mf, bf = _final_mat(p['out_w'], p['out_g'], p['out_b'], 32)
    pk.add_w('fin', _tiles_of(Mf))
    pk.add_b('fin', bf)
    pk.glin_wcols = max(off + m for (_, _, off, m) in pk.wmap['gl_in'])
    pk.gru_wcols = max(off + m for nm in ('gl_in', 'k', 'rk', 'gl_out')
                       for (_, _, off, m) in pk.wmap[nm])
    return pk


# ---------------------------------------------------------------------------
# Device kernel
# ---------------------------------------------------------------------------

def _build_and_emit(pk):
    nc = bacc.Bacc('TRN2', target_bir_lowering=False, debug=False,
                   num_devices=NCORES)

    dr = {}
    for name, shape, dt, kind in [
            ('embT', [256, BC], BF16, 'ExternalInput'),
            ('stateT', [256, BC], BF16, 'ExternalInput'),
            ('e3T', [128, BC], BF16, 'ExternalInput'),
            ('e2T', [128, BC], BF16, 'ExternalInput'),
            ('e1T', [256, BC], BF16, 'ExternalInput'),
            ('e0T', [512, BC], BF16, 'ExternalInput'),
            ('wts', [128, pk.woff], BF16, 'ExternalInput'),
            ('bias', [128, max(len(pk.bcols), 1)], F32, 'ExternalInput'),
            ('mT', [32, BC], BF16, 'ExternalOutput'),
            ('newstT', [256, BC], BF16, 'ExternalOutput')]:
        dr[name] = nc.dram_tensor(name, shape, dt, kind=kind).ap()

    wmap, bmap = pk.wmap, pk.bmap

    with contextlib.ExitStack() as ctx:
        tc = ctx.enter_context(tile.TileContext(nc, pool_alloc_mode='queue'))
        consts = ctx.enter_context(tc.tile_pool(name='consts', bufs=1))
        inp = ctx.enter_context(tc.tile_pool(name='inp', bufs=1))
        work = ctx.enter_context(tc.tile_pool(name='work', bufs=2))
        pers = ctx.enter_context(tc.tile_pool(name='pers', bufs=NCHUNK))
        outp = ctx.enter_context(tc.tile_pool(name='outp', bufs=1))
        psum = ctx.enter_context(tc.tile_pool(name='psum', bufs=4, space='PSUM'))
        cpsum = ctx.enter_context(tc.tile_pool(name='cpsum', bufs=4, space='PSUM'))

        # ---- constants ----
        nbias = max(len(pk.bcols), 1)
        wts = consts.tile([128, pk.woff], BF16, tag='wts', name='wts_sb')
        nc.scalar.dma_start(wts[:, 0:pk.glin_wcols], dr['wts'][:, 0:pk.glin_wcols])
        nc.scalar.dma_start(wts[:, pk.glin_wcols:pk.gru_wcols],
                            dr['wts'][:, pk.glin_wcols:pk.gru_wcols])
        nc.scalar.dma_start(wts[:, pk.gru_wcols:], dr['wts'][:, pk.gru_wcols:])
        bias = consts.tile([128, nbias], F32, tag='bias', name='bias_sb')
        if pk.bcols:
            nc.scalar.dma_start(bias[:], dr['bias'][:])

        def Wap(name, idx):
            kc, mc, off, m = wmap[name][idx]
            return wts[:, off:off + m]

        def Bv(name, i):
            j = bmap[name][i]
            if j is None:
                return None
            return bias[:, j:j + 1]

        def act(out, in_, func, bname=None, bi=0, psl=None):
            bv = Bv(bname, bi) if bname is not None else None
            if bv is None:
                nc.scalar.activation(out, in_, func)
            else:
                nc.scalar.activation(out, in_, func,
                                     bias=bv if psl is None else bv[psl])

        # ---- load inputs (feature-major bf16) ----
        ins = {}
        for name, nrow in [('embT', 256), ('stateT', 256), ('e3T', 128),
                           ('e2T', 128), ('e1T', 256), ('e0T', 512)]:
            ins[name] = [inp.tile([128, BC], BF16, tag=f'{name}{t}', name=f'{name}{t}')
                         for t in range(nrow // 128)]
        for c in range(NCHUNK):
            S = slice(c * NB, (c + 1) * NB)
            for name in ('embT', 'stateT'):
                for t, tl in enumerate(ins[name]):
                    nc.sync.dma_start(tl[:, S], dr[name][t * 128:(t + 1) * 128, S])
        for name in ('e3T', 'e2T', 'e1T', 'e0T'):
            for t, tl in enumerate(ins[name]):
                nc.gpsimd.dma_start(tl[:], dr[name][t * 128:(t + 1) * 128, :])

        # ---- PE warmup: dummy matmuls during the DMA head keep the HAM
        # clock-gate at 8/8 so the real matmul stream starts warm ----
        wz = consts.tile([128, NB], BF16, tag='wz', name='wz')
        nc.vector.memset(wz[:], 0.0)
        ps_w = cpsum.tile([128, NB], F32, tag='cps', name='cps')
        for i in range(24):
            nc.tensor.matmul(ps_w[:], wz[:, 0:128], wz[:], start=i == 0, stop=i == 23)

        # ---- output staging ----
        st_out = [outp.tile([128, BC], BF16, tag=f'st{t}', name=f'st{t}')
                  for t in range(2)]
        m_out = outp.tile([32, BC], BF16, tag='m', name='m_out')

        def conv_mm(name, rhs_tiles, extra_rhs=None):
            tiles = wmap[name]
            by_m = {}
            for (kc, mc, off, m) in tiles:
                by_m.setdefault(mc, []).append((kc, off, m))
            pst = {}
            for mc, lst in sorted(by_m.items()):
                mwidth = lst[0][2]
                ps = cpsum.tile([mwidth, NB], F32, tag='cps', name='cps')
                seq = []
                for kc, off, m in lst:
                    seq.append((wts[:, off:off + m], rhs_tiles[kc]))
                    if extra_rhs is not None:
                        seq.append((wts[:, off:off + m], extra_rhs[kc]))
                for i, (w, rhs) in enumerate(seq):
                    nc.tensor.matmul(ps[:], w, rhs,
                                     start=i == 0, stop=i == len(seq) - 1)
                pst[mc] = ps
            return [pst[mc] for mc in sorted(pst)]

        def relu_b(name, pst, eng, pool, tagp=None):
            outs = []
            tagp = tagp or name
            for mi, ps in enumerate(pst):
                o = pool.tile([ps.shape[0], NB], BF16, tag=f'{tagp}r{mi}',
                              name=f'{tagp}r{mi}')
                bv = Bv(name, mi)
                if eng == 'act':
                    if bv is None:
                        nc.scalar.activation(o[:], ps[:], AF.Relu)
                    else:
                        nc.scalar.activation(o[:], ps[:], AF.Relu, bias=bv)
                else:
                    if bv is None:
                        nc.vector.tensor_scalar_max(o[:], ps[:], 0.0)
                    else:
                        nc.vector.tensor_scalar(o[:], ps[:], bv, 0.0,
                                                op0=ALU.add, op1=ALU.max)
                outs.append(o)
            return outs

        P = {}   # per-chunk persistent tiles

        def phase_gru(c):
            """GRU matmuls + gate chain + the four input-side conv blocks."""
            S = slice(c * NB, (c + 1) * NB)
            hT = [ins['stateT'][t][:, S] for t in range(2)]

            ps_gl = []
            for t in range(2):
                ps = psum.tile([128, NB], F32, tag='ps', name='ps')
                nc.tensor.matmul(ps[:], Wap('gl_in', t), ins['embT'][t][:, S],
                                 start=True, stop=True)
                ps_gl.append(ps)
            xin = []
            for t in range(2):
                xt = pers.tile([128, NB], BF16, tag=f'xin{t}', name=f'xin{t}', bufs=3)
                nc.scalar.activation(xt[:], ps_gl[t][:], AF.Relu)
                xin.append(xt)

            zr = []
            for mi in range(4):
                ps = psum.tile([128, NB], F32, tag='ps', name='ps')
                for kc in range(2):
                    nc.tensor.matmul(ps[:], Wap('rk', kc * 6 + mi), hT[kc],
                                     start=kc == 0, stop=False)
                for kc in range(2):
                    nc.tensor.matmul(ps[:], Wap('k', kc * 6 + mi), xin[kc][:],
                                     start=False, stop=kc == 1)
                g = work.tile([128, NB], BF16, tag=f'zr{mi}', name=f'zr{mi}')
                act(g[:], ps[:], AF.Sigmoid, 'bzr', mi)
                zr.append(g)
            z, r = zr[0:2], zr[2:4]

            zh = []
            for t in range(2):
                zht = work.tile([128, NB], BF16, tag=f'zh{t}', name=f'zh{t}')
                nc.vector.tensor_tensor(zht[:], z[t][:], hT[t], op=ALU.mult)
                zh.append(zht)

            for t in range(2):
                mi = 4 + t
                ps_hn = psum.tile([128, NB], F32, tag='ps', name='ps')
                for kc in range(2):
                    nc.tensor.matmul(ps_hn[:], Wap('rk', kc * 6 + mi), hT[kc],
                                     start=kc == 0, stop=kc == 1)
                ps_xn = psum.tile([128, NB], F32, tag='ps', name='ps')
                for kc in range(2):
                    nc.tensor.matmul(ps_xn[:], Wap('k', kc * 6 + mi), xin[kc][:],
                                     start=kc == 0, stop=kc == 1)
                hnr = work.tile([128, NB], BF16, tag=f'hnr{t}', name=f'hnr{t}')
                b1nv = Bv('b1n', t)
                if b1nv is None:
                    nc.vector.tensor_tensor(hnr[:], ps_hn[:], r[t][:], op=ALU.mult)
                else:
                    nc.vector.scalar_tensor_tensor(hnr[:], ps_hn[:], b1nv,
                                                   r[t][:], op0=ALU.add,
                                                   op1=ALU.mult)
                nin = work.tile([128, NB], BF16, tag=f'nin{t}', name=f'nin{t}')
                nc.vector.tensor_tensor(nin[:], ps_xn[:], hnr[:], op=ALU.add)
                nt = work.tile([128, NB], BF16, tag=f'n{t}', name=f'n{t}')
                act(nt[:], nin[:], AF.Tanh, 'b0n', t)
                zn = work.tile([128, NB], BF16, tag=f'zn{t}', name=f'zn{t}')
                nc.vector.tensor_tensor(zn[:], z[t][:], nt[:], op=ALU.mult)
                nmz = work.tile([128, NB], BF16, tag=f'nmz{t}', name=f'nmz{t}')
                nc.vector.tensor_tensor(nmz[:], nt[:], zn[:], op=ALU.subtract)
                hpt = st_out[t][:, S]
                nc.vector.tensor_tensor(hpt, zh[t][:], nmz[:], op=ALU.add)

            for t in range(2):
                nc.sync.dma_start(dr['newstT'][t * 128:(t + 1) * 128, S],
                                  st_out[t][:, S])

            # input-side conv blocks (only need e*T): PE filler for gate drain
            rc3 = relu_b('c3p', conv_mm('c3p', [ins['e3T'][0][:, S]]), 'vec', pers)
            rc2 = relu_b('c2p', conv_mm('c2p', [ins['e2T'][0][:, S]]), 'vec', pers)
            rc1 = relu_b('c1p', conv_mm('c1p', [t[:, S] for t in ins['e1T']]),
                         'vec', pers)
            rc0 = relu_b('c0p', conv_mm('c0p', [t[:, S] for t in ins['e0T']]),
                         'act', pers)
            P[c] = dict(xin=xin, rc3=rc3, rc2=rc2, rc1=rc1, rc0=rc0)

        def phase_glout(c):
            """gl_out (gout = hp + xin folded as extra contraction) -> emb4."""
            S = slice(c * NB, (c + 1) * NB)
            ps = cpsum.tile([128, NB], F32, tag='cps', name='cps')
            seqg = []
            for i in range(2):
                kc = wmap['gl_out'][i][0]
                seqg.append((Wap('gl_out', i), st_out[kc][:, S]))
            for i in range(2):
                kc = wmap['gl_out'][i][0]
                seqg.append((Wap('gl_out', i), P[c]['xin'][kc][:]))
            for i, (w, rhs) in enumerate(seqg):
                nc.tensor.matmul(ps[:], w, rhs, start=i == 0,
                                 stop=i == len(seqg) - 1)
            emb4 = pers.tile([128, NB], BF16, tag='emb4', name='emb4', bufs=3)
            nc.scalar.activation(emb4[:], ps[:], AF.Relu)
            P[c]['emb4'] = emb4

        def stage_t3(c):
            P[c]['x3'] = relu_b('t3', conv_mm('t3', [P[c]['rc3'][0][:]],
                                              extra_rhs=[P[c]['emb4'][:]]),
                                'act', pers)

        def stage_t2(c):
            P[c]['x2'] = relu_b('t2', conv_mm('t2', [P[c]['rc2'][0][:]],
                                              extra_rhs=[P[c]['x3'][0][:]]),
                                'act', pers)

        def stage_t1(c):
            x1 = []
            for t in range(2):
                xt = work.tile([128, NB], BF16, tag=f'x1_{t}', name=f'x1_{t}')
                nc.vector.tensor_tensor(xt[:], P[c]['rc1'][t][:], P[c]['x2'][t][:],
                                        op=ALU.add)
                x1.append(xt)
            P[c]['x1t'] = relu_b('t1', conv_mm('t1', [x[:] for x in x1]),
                                 'vec', pers)

        def stage_fin(c):
            S = slice(c * NB, (c + 1) * NB)
            psf = cpsum.tile([32, NB], F32, tag='cps', name='cps')
            seqf = []
            for (kc, mc, off, m) in wmap['fin']:
                seqf.append((wts[:, off:off + m], P[c]['rc0'][kc][:]))
                seqf.append((wts[:, off:off + m], P[c]['x1t'][kc][:]))
            for i, (w, rhs) in enumerate(seqf):
                nc.tensor.matmul(psf[:], w, rhs, start=i == 0,
                                 stop=i == len(seqf) - 1)
            act(m_out[:, S], psf[:], AF.Sigmoid, 'fin', 0, psl=slice(0, 32))
            nc.sync.dma_start(dr['mT'][:, S], m_out[:, S])

        # Hand-pipelined schedule: glout(c) is skewed two GRU chunks later so
        # the gate chain of chunk c drains while PE runs chunks c+1, c+2.
        phase_gru(0)
        phase_gru(1)
        phase_glout(0)
        phase_gru(2)
        phase_glout(1)
        phase_gru(3)
        phase_glout(2)
        stage_t3(0)
        phase_glout(3)
        stage_t3(1)
        stage_t2(0)
        stage_t3(2)
        stage_t2(1)
        stage_t3(3)
        stage_t2(2)
        stage_t1(0)
        stage_t2(3)
        stage_t1(1)
        stage_fin(0)
        stage_t1(2)
        stage_fin(1)
        stage_t1(3)
        stage_fin(2)
        stage_fin(3)

    nc.compile()
    return nc


# ---------------------------------------------------------------------------
# Public entry point
# ---------------------------------------------------------------------------

def make_in_maps(emb, e3, e2, e1, e0, erb_dec_state, wts, bias):
    bf = ml_dtypes.bfloat16
    embT = np.ascontiguousarray(np.asarray(emb, np.float32).T).astype(bf)
    stT = np.ascontiguousarray(np.asarray(erb_dec_state, np.float32).T).astype(bf)
    e3T = np.ascontiguousarray(np.asarray(e3, np.float32).reshape(B, -1).T).astype(bf)
    e2T = np.ascontiguousarray(np.asarray(e2, np.float32).reshape(B, -1).T).astype(bf)
    e1T = np.ascontiguousarray(np.asarray(e1, np.float32).reshape(B, -1).T).astype(bf)
    e0T = np.ascontiguousarray(np.asarray(e0, np.float32).reshape(B, -1).T).astype(bf)
    in_maps = []
    for i in range(NCORES):
        S = slice(i * BC, (i + 1) * BC)
        in_maps.append({
            'embT': np.ascontiguousarray(embT[:, S]),
            'stateT': np.ascontiguousarray(stT[:, S]),
            'e3T': np.ascontiguousarray(e3T[:, S]),
            'e2T': np.ascontiguousarray(e2T[:, S]),
            'e1T': np.ascontiguousarray(e1T[:, S]),
            'e0T': np.ascontiguousarray(e0T[:, S]),
            'wts': wts,
            'bias': bias,
        })
    return in_maps


def _ensure_ntff_hook():
    """Register the axon NTFF profiling hook if the image's antenv lacks it."""
    try:
        from antenv.axon_hooks import get_axon_ntff_profile_hook  # noqa: F401
        return
    except ImportError:
        pass
    import types

    import antenv
    mod = types.ModuleType('antenv.axon_hooks')
    _h = [None]
    mod.get_axon_ntff_profile_hook = lambda: _h[0]
    mod.set_axon_ntff_profile_hook = lambda hook: _h.__setitem__(0, hook)
    sys.modules['antenv.axon_hooks'] = mod
    antenv.axon_hooks = mod
    try:
        from trn_agent_boot.trn_boot import _ntff_profile_via_ctypes
        mod.set_axon_ntff_profile_hook(
            _ntff_profile_via_ctypes('/opt/axon/libaxon_pjrt.so'))
    except Exception:
        pass


def kernel(emb, e3, e2, e1, e0, erb_dec_state, params):
    global LAST_RESULTS
    if os.environ.get('BASS_TRACE'):
        _ensure_ntff_hook()
    pk = _prepare(params)
    wts, bias = pk.finalize()
    nc = _build_and_emit(pk)
    in_maps = make_in_maps(emb, e3, e2, e1, e0, erb_dec_state, wts, bias)

    res = run_bass_kernel_spmd(nc, in_maps, list(range(NCORES)),
                               trace=bool(os.environ.get('BASS_TRACE')))
    LAST_RESULTS = res

    m = np.empty((B, 32), np.float32)
    new_state = np.empty((B, 256), np.float32)
    for i in range(NCORES):
        S = slice(i * BC, (i + 1) * BC)
        m[S] = np.asarray(res.results[i]['mT']).astype(np.float32).T
        new_state[S] = np.asarray(res.results[i]['newstT']).astype(np.float32).T
    return m.reshape(B, 1, 1, 32), new_state


# revision 37
# speedup vs baseline: 1.0351x; 1.0098x over previous
"""Trainium2 Bass kernel for nn_ErbDecoderStep (GRU + grouped-linear + separable-conv decoder).

Strategy:
  - Pure data parallel: batch B=16384 sharded as 2048 rows per NeuronCore (8 cores).
  - All device compute is feature-major: activations live in SBUF as (features, batch)
    tiles so every matmul contracts over the partition axis. The host pre-transposes
    (and casts to bf16) the inputs and transposes the two outputs back.
  - Depthwise+pointwise+folded-BN of every separable conv block is combined on the host
    into one small banded matrix per block (transpose convs become banded upsampling
    matrices); residual adds are folded into the following matmul as an extra
    contraction block reusing the same weight tiles, or done on DVE.
  - bf16 matmuls with fp32 PSUM accumulation; bf16 DRAM outputs upcast on host.
  - Hand-pipelined schedule: per-chunk GRU groups (PSUM bank released per gate via
    [rk,rk,k,k]+sigmoid), gl_out skewed two chunks later so the gate chain drains off
    the PE critical path, then stage-major conv rounds (t3/t2/t1/fin across chunks).
  - DMA: weights/bias on the ACT HWDGE ring, emb/state chunk-sliced on the SP ring,
    e3..e0 via SWDGE; stores ride SP after all loads (HWDGE rings are FIFO - a store
    that waits on compute must never sit ahead of a load).

Self-contained: hardcodes shapes from the problem spec (B=16384, H=256, ch=16, G=8).
"""
import contextlib
import os
import sys

import numpy as np

for _p in ('/opt/trn_rl_repo', '/root/.axon_site/_ro/trn_rl_repo'):
    if _p not in sys.path and os.path.isdir(_p):
        sys.path.insert(0, _p)

import ml_dtypes  # noqa: E402

import concourse.bacc as bacc  # noqa: E402
import concourse.bass as bass  # noqa: E402
import concourse.mybir as mybir  # noqa: E402
import concourse.tile as tile  # noqa: E402
from concourse.bass_utils import run_bass_kernel_spmd  # noqa: E402

BF16 = mybir.dt.bfloat16
F32 = mybir.dt.float32
AF = mybir.ActivationFunctionType
ALU = mybir.AluOpType

B = 16384
NCORES = 8
BC = B // NCORES          # 2048 batch rows per core
NCHUNK = 4
NB = BC // NCHUNK         # 512 batch columns per chunk (one PSUM bank per f32 tile)
CH = 16

LAST_RESULTS = None       # test.py reads profiling info from here

# ---------------------------------------------------------------------------
# Host-side weight preprocessing
# ---------------------------------------------------------------------------


def _sep_mats(dw, pw, g, b, F_in, mode):
    """Combined depthwise+pointwise+BN-gain matrix (F_in*ch, F_out*ch) + bias vec."""
    dw = np.asarray(dw, np.float32)
    pw = np.asarray(pw, np.float32)
    g = np.asarray(g, np.float32)
    b = np.asarray(b, np.float32)
    kf = dw.shape[1]
    ch = dw.shape[3]
    dwv = dw[0, :, 0, :]
    pwm = pw[0, 0]
    if mode == 'same':
        F_out = F_in
        M = np.zeros((F_in * ch, F_out * ch), np.float32)
        off = 1 if kf == 3 else 0
        for fo in range(F_out):
            for t in range(kf):
                fi = fo + t - off
                if 0 <= fi < F_in:
                    M[fi * ch:(fi + 1) * ch, fo * ch:(fo + 1) * ch] += dwv[t][:, None] * pwm
    else:  # 'tr': lhs_dilation 2, pad (1,2), kf=3
        F_out = 2 * F_in
        M = np.zeros((F_in * ch, F_out * ch), np.float32)
        for fo in range(F_out):
            for t in range(3):
                j = fo + t - 1
                if j >= 0 and j % 2 == 0 and j // 2 < F_in:
                    fi = j // 2
                    M[fi * ch:(fi + 1) * ch, fo * ch:(fo + 1) * ch] += dwv[t][:, None] * pwm
    M = M * np.tile(g, F_out)[None, :]
    return M, np.tile(b, F_out)


def _final_mat(out_w, out_g, out_b, F=32):
    out_w = np.asarray(out_w, np.float32)
    ch = out_w.shape[2]
    M = np.zeros((F * ch, F), np.float32)
    for fo in range(F):
        for t in range(3):
            fi = fo + t - 1
            if 0 <= fi < F:
                M[fi * ch:(fi + 1) * ch, fo] += out_w[0, t, :, 0]
    M = M * float(np.asarray(out_g).reshape(-1)[0])
    return M, np.full((F,), float(np.asarray(out_b).reshape(-1)[0]), np.float32)


def _grouped_bd(w):
    w = np.asarray(w, np.float32)
    gn, ig, og = w.shape
    M = np.zeros((gn * ig, gn * og), np.float32)
    for gi in range(gn):
        M[gi * ig:(gi + 1) * ig, gi * og:(gi + 1) * og] = w[gi]
    return M


def _tiles_of(M):
    """Split (K, Mo) into 128x<=128 lhsT tiles, dropping all-zero tiles."""
    K, Mo = M.shape
    out = []
    for kc in range(0, K, 128):
        for mc in range(0, Mo, 128):
            t = M[kc:kc + 128, mc:mc + 128]
            if not np.any(t):
                continue
            out.append((kc // 128, mc // 128, np.ascontiguousarray(t)))
    return out


class _WeightPack:
    def __init__(self):
        self.wcols = []
        self.woff = 0
        self.wmap = {}     # name -> list of (kc, mc, col_offset, width)
        self.bcols = []
        self.bmap = {}     # name -> list of column indices (one per 128-row tile)

    def add_w(self, name, tiles):
        lst = []
        for kc, mc, t in tiles:
            k, m = t.shape
            buf = np.zeros((128, m), np.float32)
            buf[:k] = t
            lst.append((kc, mc, self.woff, m))
            self.wcols.append(buf)
            self.woff += m
        self.wmap[name] = lst

    def add_b(self, name, vec):
        vec = np.asarray(vec, np.float32).reshape(-1)
        if not np.any(vec):
            self.bmap[name] = [None] * ((len(vec) + 127) // 128)
            return
        cols = []
        for mc in range(0, len(vec), 128):
            seg = vec[mc:mc + 128]
            buf = np.zeros((128,), np.float32)
            buf[:len(seg)] = seg
            cols.append(len(self.bcols))
            self.bcols.append(buf)
        self.bmap[name] = cols

    def finalize(self):
        wts = np.concatenate(self.wcols, axis=1).astype(ml_dtypes.bfloat16)
        if self.bcols:
            bias = np.stack(self.bcols, axis=1).astype(np.float32)
        else:
            bias = np.zeros((128, 1), np.float32)
        return wts, bias


def _prepare(params):
    p = {k: np.asarray(v, np.float32) for k, v in params.items()}
    pk = _WeightPack()

    pk.add_w('gl_in', _tiles_of(_grouped_bd(p['w_in'])))
    pk.add_w('k', _tiles_of(p['gru_k']))
    pk.add_w('rk', _tiles_of(p['gru_rk']))
    pk.add_w('gl_out', _tiles_of(_grouped_bd(p['w_out'])))

    b0, b1 = p['gru_b'][0], p['gru_b'][1]
    pk.add_b('bzr', (b0 + b1)[0:512])
    pk.add_b('b0n', b0[512:768])
    pk.add_b('b1n', b1[512:768])

    for name, fin, mode in [('c3p', 8, 'same'), ('t3', 8, 'same'),
                            ('c2p', 8, 'same'), ('t2', 8, 'tr'),
                            ('c1p', 16, 'same'), ('t1', 16, 'tr'),
                            ('c0p', 32, 'same')]:
        M, bv = _sep_mats(p[name + '_dw'], p[name + '_pw'], p[name + '_g'],
                          p[name + '_b'], fin, mode)
        pk.add_w(name, _tiles_of(M))
        pk.add_b(name, bv)

    Mf, bf = _final_mat(p['out_w'], p['out_g'], p['out_b'], 32)
    pk.add_w('fin', _tiles_of(Mf))
    pk.add_b('fin', bf)
    pk.glin_wcols = max(off + m for (_, _, off, m) in pk.wmap['gl_in'])
    pk.gru_wcols = max(off + m for nm in ('gl_in', 'k', 'rk', 'gl_out')
                       for (_, _, off, m) in pk.wmap[nm])
    return pk


# ---------------------------------------------------------------------------
# Device kernel
# ---------------------------------------------------------------------------

def _build_and_emit(pk):
    nc = bacc.Bacc('TRN2', target_bir_lowering=False, debug=False,
                   num_devices=NCORES)

    dr = {}
    for name, shape, dt, kind in [
            ('embT', [256, BC], BF16, 'ExternalInput'),
            ('stateT', [256, BC], BF16, 'ExternalInput'),
            ('e3T', [128, BC], BF16, 'ExternalInput'),
            ('e2T', [128, BC], BF16, 'ExternalInput'),
            ('e1T', [256, BC], BF16, 'ExternalInput'),
            ('e0T', [512, BC], BF16, 'ExternalInput'),
            ('wts', [128, pk.woff], BF16, 'ExternalInput'),
            ('bias', [128, max(len(pk.bcols), 1)], F32, 'ExternalInput'),
            ('mT', [32, BC], BF16, 'ExternalOutput'),
            ('newstT', [256, BC], BF16, 'ExternalOutput')]:
        dr[name] = nc.dram_tensor(name, shape, dt, kind=kind).ap()

    wmap, bmap = pk.wmap, pk.bmap

    with contextlib.ExitStack() as ctx:
        tc = ctx.enter_context(tile.TileContext(nc))
        consts = ctx.enter_context(tc.tile_pool(name='consts', bufs=1))
        inp = ctx.enter_context(tc.tile_pool(name='inp', bufs=1))
        work = ctx.enter_context(tc.tile_pool(name='work', bufs=2))
        pers = ctx.enter_context(tc.tile_pool(name='pers', bufs=NCHUNK))
        outp = ctx.enter_context(tc.tile_pool(name='outp', bufs=1))
        psum = ctx.enter_context(tc.tile_pool(name='psum', bufs=4, space='PSUM'))
        cpsum = ctx.enter_context(tc.tile_pool(name='cpsum', bufs=4, space='PSUM'))

        # ---- constants ----
        nbias = max(len(pk.bcols), 1)
        wts = consts.tile([128, pk.woff], BF16, tag='wts', name='wts_sb')
        nc.scalar.dma_start(wts[:, 0:pk.glin_wcols], dr['wts'][:, 0:pk.glin_wcols])
        nc.scalar.dma_start(wts[:, pk.glin_wcols:pk.gru_wcols],
                            dr['wts'][:, pk.glin_wcols:pk.gru_wcols])
        nc.scalar.dma_start(wts[:, pk.gru_wcols:], dr['wts'][:, pk.gru_wcols:])
        bias = consts.tile([128, nbias], F32, tag='bias', name='bias_sb')
        if pk.bcols:
            nc.scalar.dma_start(bias[:], dr['bias'][:])

        def Wap(name, idx):
            kc, mc, off, m = wmap[name][idx]
            return wts[:, off:off + m]

        def Bv(name, i):
            j = bmap[name][i]
            if j is None:
                return None
            return bias[:, j:j + 1]

        def act(out, in_, func, bname=None, bi=0, psl=None):
            bv = Bv(bname, bi) if bname is not None else None
            if bv is None:
                nc.scalar.activation(out, in_, func)
            else:
                nc.scalar.activation(out, in_, func,
                                     bias=bv if psl is None else bv[psl])

        # ---- load inputs (feature-major bf16) ----
        ins = {}
        for name, nrow in [('embT', 256), ('stateT', 256), ('e3T', 128),
                           ('e2T', 128), ('e1T', 256), ('e0T', 512)]:
            ins[name] = [inp.tile([128, BC], BF16, tag=f'{name}{t}', name=f'{name}{t}')
                         for t in range(nrow // 128)]
        for c in range(NCHUNK):
            S = slice(c * NB, (c + 1) * NB)
            for name in ('embT', 'stateT'):
                for t, tl in enumerate(ins[name]):
                    nc.sync.dma_start(tl[:, S], dr[name][t * 128:(t + 1) * 128, S])
        for name in ('e3T', 'e2T', 'e1T', 'e0T'):
            for t, tl in enumerate(ins[name]):
                nc.gpsimd.dma_start(tl[:], dr[name][t * 128:(t + 1) * 128, :])

        # ---- PE warmup: dummy matmuls during the DMA head keep the HAM
        # clock-gate at 8/8 so the real matmul stream starts warm ----
        wz = consts.tile([128, NB], BF16, tag='wz', name='wz')
        nc.vector.memset(wz[:], 0.0)
        ps_w = cpsum.tile([128, NB], F32, tag='cps', name='cps')
        for i in range(24):
            nc.tensor.matmul(ps_w[:], wz[:, 0:128], wz[:], start=i == 0, stop=i == 23)

        # ---- output staging ----
        st_out = [outp.tile([128, BC], BF16, tag=f'st{t}', name=f'st{t}')
                  for t in range(2)]
        m_out = outp.tile([32, BC], BF16, tag='m', name='m_out')

        def conv_mm(name, rhs_tiles, extra_rhs=None):
            tiles = wmap[name]
            by_m = {}
            for (kc, mc, off, m) in tiles:
                by_m.setdefault(mc, []).append((kc, off, m))
            pst = {}
            for mc, lst in sorted(by_m.items()):
                mwidth = lst[0][2]
                ps = cpsum.tile([mwidth, NB], F32, tag='cps', name='cps')
                seq = []
                for kc, off, m in lst:
                    seq.append((wts[:, off:off + m], rhs_tiles[kc]))
                    if extra_rhs is not None:
                        seq.append((wts[:, off:off + m], extra_rhs[kc]))
                for i, (w, rhs) in enumerate(seq):
                    nc.tensor.matmul(ps[:], w, rhs,
                                     start=i == 0, stop=i == len(seq) - 1)
                pst[mc] = ps
            return [pst[mc] for mc in sorted(pst)]

        def relu_b(name, pst, eng, pool, tagp=None):
            outs = []
            tagp = tagp or name
            for mi, ps in enumerate(pst):
                o = pool.tile([ps.shape[0], NB], BF16, tag=f'{tagp}r{mi}',
                              name=f'{tagp}r{mi}')
                bv = Bv(name, mi)
                if eng == 'act':
                    if bv is None:
                        nc.scalar.activation(o[:], ps[:], AF.Relu)
                    else:
                        nc.scalar.activation(o[:], ps[:], AF.Relu, bias=bv)
                else:
                    if bv is None:
                        nc.vector.tensor_scalar_max(o[:], ps[:], 0.0)
                    else:
                        nc.vector.tensor_scalar(o[:], ps[:], bv, 0.0,
                                                op0=ALU.add, op1=ALU.max)
                outs.append(o)
            return outs

        P = {}   # per-chunk persistent tiles

        def phase_gru(c):
            """GRU matmuls + gate chain + the four input-side conv blocks."""
            S = slice(c * NB, (c + 1) * NB)
            hT = [ins['stateT'][t][:, S] for t in range(2)]

            ps_gl = []
            for t in range(2):
                ps = psum.tile([128, NB], F32, tag='ps', name='ps')
                nc.tensor.matmul(ps[:], Wap('gl_in', t), ins['embT'][t][:, S],
                                 start=True, stop=True)
                ps_gl.append(ps)
            xin = []
            for t in range(2):
                xt = pers.tile([128, NB], BF16, tag=f'xin{t}', name=f'xin{t}', bufs=3)
                nc.scalar.activation(xt[:], ps_gl[t][:], AF.Relu)
                xin.append(xt)

            zr = []
            for mi in range(4):
                ps = psum.tile([128, NB], F32, tag='ps', name='ps')
                for kc in range(2):
                    nc.tensor.matmul(ps[:], Wap('rk', kc * 6 + mi), hT[kc],
                                     start=kc == 0, stop=False)
                for kc in range(2):
                    nc.tensor.matmul(ps[:], Wap('k', kc * 6 + mi), xin[kc][:],
                                     start=False, stop=kc == 1)
                g = work.tile([128, NB], BF16, tag=f'zr{mi}', name=f'zr{mi}')
                act(g[:], ps[:], AF.Sigmoid, 'bzr', mi)
                zr.append(g)
            z, r = zr[0:2], zr[2:4]

            zh = []
            for t in range(2):
                zht = work.tile([128, NB], BF16, tag=f'zh{t}', name=f'zh{t}')
                nc.vector.tensor_tensor(zht[:], z[t][:], hT[t], op=ALU.mult)
                zh.append(zht)

            for t in range(2):
                mi = 4 + t
                ps_hn = psum.tile([128, NB], F32, tag='ps', name='ps')
                for kc in range(2):
                    nc.tensor.matmul(ps_hn[:], Wap('rk', kc * 6 + mi), hT[kc],
                                     start=kc == 0, stop=kc == 1)
                ps_xn = psum.tile([128, NB], F32, tag='ps', name='ps')
                for kc in range(2):
                    nc.tensor.matmul(ps_xn[:], Wap('k', kc * 6 + mi), xin[kc][:],
                                     start=kc == 0, stop=kc == 1)
                hnr = work.tile([128, NB], BF16, tag=f'hnr{t}', name=f'hnr{t}')
                b1nv = Bv('b1n', t)
                if b1nv is None:
                    nc.vector.tensor_tensor(hnr[:], ps_hn[:], r[t][:], op=ALU.mult)
                else:
                    nc.vector.scalar_tensor_tensor(hnr[:], ps_hn[:], b1nv,
                                                   r[t][:], op0=ALU.add,
                                                   op1=ALU.mult)
                nin = work.tile([128, NB], BF16, tag=f'nin{t}', name=f'nin{t}')
                nc.vector.tensor_tensor(nin[:], ps_xn[:], hnr[:], op=ALU.add)
                nt = work.tile([128, NB], BF16, tag=f'n{t}', name=f'n{t}')
                act(nt[:], nin[:], AF.Tanh, 'b0n', t)
                zn = work.tile([128, NB], BF16, tag=f'zn{t}', name=f'zn{t}')
                nc.vector.tensor_tensor(zn[:], z[t][:], nt[:], op=ALU.mult)
                nmz = work.tile([128, NB], BF16, tag=f'nmz{t}', name=f'nmz{t}')
                nc.vector.tensor_tensor(nmz[:], nt[:], zn[:], op=ALU.subtract)
                hpt = st_out[t][:, S]
                nc.vector.tensor_tensor(hpt, zh[t][:], nmz[:], op=ALU.add)

            for t in range(2):
                nc.sync.dma_start(dr['newstT'][t * 128:(t + 1) * 128, S],
                                  st_out[t][:, S])

            # input-side conv blocks (only need e*T): PE filler for gate drain
            rc3 = relu_b('c3p', conv_mm('c3p', [ins['e3T'][0][:, S]]), 'vec', pers)
            rc2 = relu_b('c2p', conv_mm('c2p', [ins['e2T'][0][:, S]]), 'vec', pers)
            rc1 = relu_b('c1p', conv_mm('c1p', [t[:, S] for t in ins['e1T']]),
                         'vec', pers)
            rc0 = relu_b('c0p', conv_mm('c0p', [t[:, S] for t in ins['e0T']]),
                         'act', pers)
            P[c] = dict(xin=xin, rc3=rc3, rc2=rc2, rc1=rc1, rc0=rc0)

        def phase_glout(c):
            """gl_out (gout = hp + xin folded as extra contraction) -> emb4."""
            S = slice(c * NB, (c + 1) * NB)
            ps = cpsum.tile([128, NB], F32, tag='cps', name='cps')
            seqg = []
            for i in range(2):
                kc = wmap['gl_out'][i][0]
                seqg.append((Wap('gl_out', i), st_out[kc][:, S]))
            for i in range(2):
                kc = wmap['gl_out'][i][0]
                seqg.append((Wap('gl_out', i), P[c]['xin'][kc][:]))
            for i, (w, rhs) in enumerate(seqg):
                nc.tensor.matmul(ps[:], w, rhs, start=i == 0,
                                 stop=i == len(seqg) - 1)
            emb4 = pers.tile([128, NB], BF16, tag='emb4', name='emb4', bufs=3)
            nc.scalar.activation(emb4[:], ps[:], AF.Relu)
            P[c]['emb4'] = emb4

        def stage_t3(c):
            P[c]['x3'] = relu_b('t3', conv_mm('t3', [P[c]['rc3'][0][:]],
                                              extra_rhs=[P[c]['emb4'][:]]),
                                'act', pers)

        def stage_t2(c):
            P[c]['x2'] = relu_b('t2', conv_mm('t2', [P[c]['rc2'][0][:]],
                                              extra_rhs=[P[c]['x3'][0][:]]),
                                'act', pers)

        def stage_t1(c):
            x1 = []
            for t in range(2):
                xt = work.tile([128, NB], BF16, tag=f'x1_{t}', name=f'x1_{t}')
                nc.vector.tensor_tensor(xt[:], P[c]['rc1'][t][:], P[c]['x2'][t][:],
                                        op=ALU.add)
                x1.append(xt)
            P[c]['x1t'] = relu_b('t1', conv_mm('t1', [x[:] for x in x1]),
                                 'vec', pers)

        def stage_fin(c):
            S = slice(c * NB, (c + 1) * NB)
            psf = cpsum.tile([32, NB], F32, tag='cps', name='cps')
            seqf = []
            for (kc, mc, off, m) in wmap['fin']:
                seqf.append((wts[:, off:off + m], P[c]['rc0'][kc][:]))
                seqf.append((wts[:, off:off + m], P[c]['x1t'][kc][:]))
            for i, (w, rhs) in enumerate(seqf):
                nc.tensor.matmul(psf[:], w, rhs, start=i == 0,
                                 stop=i == len(seqf) - 1)
            act(m_out[:, S], psf[:], AF.Sigmoid, 'fin', 0, psl=slice(0, 32))
            nc.sync.dma_start(dr['mT'][:, S], m_out[:, S])

        # Hand-pipelined schedule: glout(c) is skewed two GRU chunks later so
        # the gate chain of chunk c drains while PE runs chunks c+1, c+2.
        phase_gru(0)
        phase_gru(1)
        phase_glout(0)
        phase_gru(2)
        phase_glout(1)
        phase_gru(3)
        phase_glout(2)
        stage_t3(0)
        phase_glout(3)
        stage_t3(1)
        stage_t2(0)
        stage_t3(2)
        stage_t2(1)
        stage_t3(3)
        stage_t2(2)
        stage_t1(0)
        stage_t2(3)
        stage_t1(1)
        stage_fin(0)
        stage_t1(2)
        stage_fin(1)
        stage_t1(3)
        stage_fin(2)
        stage_fin(3)

    nc.compile()
    return nc


# ---------------------------------------------------------------------------
# Public entry point
# ---------------------------------------------------------------------------

def make_in_maps(emb, e3, e2, e1, e0, erb_dec_state, wts, bias):
    bf = ml_dtypes.bfloat16
    embT = np.ascontiguousarray(np.asarray(emb, np.float32).T).astype(bf)
    stT = np.ascontiguousarray(np.asarray(erb_dec_state, np.float32).T).astype(bf)
    e3T = np.ascontiguousarray(np.asarray(e3, np.float32).reshape(B, -1).T).astype(bf)
    e2T = np.ascontiguousarray(np.asarray(e2, np.float32).reshape(B, -1).T).astype(bf)
    e1T = np.ascontiguousarray(np.asarray(e1, np.float32).reshape(B, -1).T).astype(bf)
    e0T = np.ascontiguousarray(np.asarray(e0, np.float32).reshape(B, -1).T).astype(bf)
    in_maps = []
    for i in range(NCORES):
        S = slice(i * BC, (i + 1) * BC)
        in_maps.append({
            'embT': np.ascontiguousarray(embT[:, S]),
            'stateT': np.ascontiguousarray(stT[:, S]),
            'e3T': np.ascontiguousarray(e3T[:, S]),
            'e2T': np.ascontiguousarray(e2T[:, S]),
            'e1T': np.ascontiguousarray(e1T[:, S]),
            'e0T': np.ascontiguousarray(e0T[:, S]),
            'wts': wts,
            'bias': bias,
        })
    return in_maps


def _ensure_ntff_hook():
    """Register the axon NTFF profiling hook if the image's antenv lacks it."""
    try:
        from antenv.axon_hooks import get_axon_ntff_profile_hook  # noqa: F401
        return
    except ImportError:
        pass
    import types

    import antenv
    mod = types.ModuleType('antenv.axon_hooks')
    _h = [None]
    mod.get_axon_ntff_profile_hook = lambda: _h[0]
    mod.set_axon_ntff_profile_hook = lambda hook: _h.__setitem__(0, hook)
    sys.modules['antenv.axon_hooks'] = mod
    antenv.axon_hooks = mod
    try:
        from trn_agent_boot.trn_boot import _ntff_profile_via_ctypes
        mod.set_axon_ntff_profile_hook(
            _ntff_profile_via_ctypes('/opt/axon/libaxon_pjrt.so'))
    except Exception:
        pass


def kernel(emb, e3, e2, e1, e0, erb_dec_state, params):
    global LAST_RESULTS
    if os.environ.get('BASS_TRACE'):
        _ensure_ntff_hook()
    pk = _prepare(params)
    wts, bias = pk.finalize()
    nc = _build_and_emit(pk)
    in_maps = make_in_maps(emb, e3, e2, e1, e0, erb_dec_state, wts, bias)

    res = run_bass_kernel_spmd(nc, in_maps, list(range(NCORES)),
                               trace=bool(os.environ.get('BASS_TRACE')))
    LAST_RESULTS = res

    m = np.empty((B, 32), np.float32)
    new_state = np.empty((B, 256), np.float32)
    for i in range(NCORES):
        S = slice(i * BC, (i + 1) * BC)
        m[S] = np.asarray(res.results[i]['mT']).astype(np.float32).T
        new_state[S] = np.asarray(res.results[i]['newstT']).astype(np.float32).T
    return m.reshape(B, 1, 1, 32), new_state
